# revision 1
# baseline (speedup 1.0000x reference)
"""Trainium2 Bass kernel for nn_DiffusionActionHead (B=8, S=2048, D=4096).

Strategy (8 NeuronCores):
  - Data-parallel over batch for everything touching llm_output (32 MiB/core).
  - Tensor-parallel weight reads: core i reads column-slice i of wq/wk/wv,
    row-slice i of wo, column/row slice i of mlp_w1/mlp_w2 (~96 MiB of
    weights split 8 ways), tiny diffusion tail replicated.
  - MAP-head attention with q_len=1 is collapsed algebraically:
        scores[s,h] = llm[s,:] . U[:,h],   U[:,h] = wk[:,hb] @ q_h / sqrt(DH)
        pooled[h,:] = softmax(scores)[h,:] @ llm
        ctx[hb]     = pooled[h,:] @ wv[:,hb] + bv[hb]
    (bk shifts scores by a per-head constant -> cancels in softmax.)
  - 4 small collectives: AllGather(U cols), AllToAll(pooled, head<->batch),
    AllReduce(attn_out partial), AllReduce(mlp partial).
  - Large matmuls run in fp16 (1 cyc/row on PE, half the HBM bytes); all
    accumulation, softmax, layernorms and residuals stay fp32.
  - Biases are folded into the PSUM accumulations via a ones-row matmul;
    additive biases of AllReduce'd partials are pre-divided by 8 on host.
  - Activations are kept feature-on-partition ("transposed") so every big
    matmul streams its weight slice in natural DRAM layout as the moving
    operand; llm itself is passed in both layouts (llmT host-transposed).
  - Two HWDGE queues: sync carries the llm streams, scalar carries the
    weight streams, so a stalled stream never head-of-line-blocks the other.
"""

import numpy as np
import sys

if "/opt/trn_rl_repo" not in sys.path:
    sys.path.insert(0, "/opt/trn_rl_repo")

import concourse.bass as bass
import concourse.tile as tile
from concourse import bacc, mybir
from concourse.masks import make_identity
from concourse.bass_utils import run_bass_kernel_spmd

F32 = mybir.dt.float32
F16 = mybir.dt.float16
AF = mybir.ActivationFunctionType
ALU = mybir.AluOpType

B, S, D = 8, 2048, 4096
H, AD, TD, HID, NBLK = 8, 7, 32, 256, 3
DH = D // H            # 512
NC = 8                 # cores
P = 128
SC = S // P            # 16 S-chunks
DC = D // P            # 32 D-chunks
HD2 = D // 2           # 2048 (half width -> 4-bank PSUM tiles)
F1S = 4 * D // NC      # 2048 per-core hidden cols of mlp_w1
HC = HID // P          # 2
RSQRT_DH = 1.0 / float(np.sqrt(DH))
TWO_PI = 2.0 * float(np.pi)


def _bcast(src_ap, nparts):
    """Partition-broadcast a (1, N) DRAM AP to (nparts, N)."""
    ap = src_ap
    assert ap.shape[0] == 1, ap.shape
    return bass.AP(tensor=ap.tensor, offset=ap.offset,
                   ap=[[0, nparts]] + [list(x) for x in ap.ap[1:]])


def build_program():
    nc = bacc.Bacc("TRN2", target_bir_lowering=False, debug=False,
                   num_devices=NC)
    t = {}

    def din(name, shape, dtype=F32):
        t[name] = nc.dram_tensor(name, shape, dtype, kind="ExternalInput")

    din("llm", [S, D], F16); din("llmT", [D, S], F16)
    din("wq_s", [D, DH], F16); din("bq_s", [1, DH])
    din("wkT_s", [DH, D], F16)
    din("wv_s", [D, DH], F16); din("bv16", [1, DH], F16)
    din("wo_s", [DH, D], F16); din("bo16", [1, D], F16)        # bo/8
    din("ln_g_r", [P, DC]); din("ln_b_r", [P, DC])
    din("w1_s", [D, F1S], F16); din("b116", [1, F1S], F16)
    din("w2_s", [F1S, D], F16); din("b216", [1, D], F16)       # b2/8
    din("probe_r", [P, DC], F16)
    din("four_w2", [TD, 1]); din("phase2", [TD, 1])
    din("timeT", [1, B]); din("naT", [AD, B], F16)
    din("cond_w1", [TD, 2 * TD], F16); din("cond_b1c", [2 * TD, 1])
    din("cond_w2", [2 * TD, TD], F16); din("cond_b2c", [TD, 1])
    din("rin_cond", [TD, HID], F16); din("rin_pool", [D, HID], F16)
    din("rin_na", [AD, HID], F16); din("rb16", [1, HID], F16)
    din("blk_g_r", [NBLK, P, HC]); din("blk_b_r", [NBLK, P, HC])
    din("blk_w1", [NBLK, HID, 4 * HID], F16)
    din("blk_b1_16", [NBLK, 4 * HID], F16)
    din("blk_w2", [NBLK, 4 * HID, HID], F16)
    din("blk_b2_16", [NBLK, HID], F16)
    din("out_w", [HID, AD], F16); din("out_bc", [1, AD])
    t["res"] = nc.dram_tensor("res", [B, AD], F32, kind="ExternalOutput")

    # collective bounce buffers (internal DRAM; AG/AR outputs in Shared space)
    t["cc_u_in"] = nc.dram_tensor("cc_u_in", [1, D], F32)
    t["cc_u_out"] = nc.dram_tensor("cc_u_out", [NC, D], F32, addr_space="Shared")
    t["cc_pool_in"] = nc.dram_tensor("cc_pool_in", [H, D], F32)
    t["cc_pool_out"] = nc.dram_tensor("cc_pool_out", [B, D], F32)
    t["cc_attn_in"] = nc.dram_tensor("cc_attn_in", [B, D], F32)
    t["cc_attn_out"] = nc.dram_tensor("cc_attn_out", [B, D], F32,
                                      addr_space="Shared")
    t["cc_mlp_in"] = nc.dram_tensor("cc_mlp_in", [B, D], F32)
    t["cc_mlp_out"] = nc.dram_tensor("cc_mlp_out", [B, D], F32,
                                     addr_space="Shared")

    with tile.TileContext(nc) as tc:
        import contextlib
        with contextlib.ExitStack() as ctx:
            _build(nc, tc, t, ctx)
    nc.finalize()
    return nc


def _build(nc, tc, t, ctx):
    GROUPS = [list(range(NC))]

    singles = ctx.enter_context(tc.tile_pool(name="singles", bufs=1))
    llm_pool = ctx.enter_context(tc.tile_pool(name="llm_pool", bufs=6))
    llmT_pool = ctx.enter_context(tc.tile_pool(name="llmT_pool", bufs=8))
    wst = ctx.enter_context(tc.tile_pool(name="wst", bufs=8))
    nat16 = ctx.enter_context(tc.tile_pool(name="nat16", bufs=2))
    nat8 = ctx.enter_context(tc.tile_pool(name="nat8", bufs=2))
    psA = ctx.enter_context(tc.tile_pool(name="psA", bufs=1, space="PSUM"))
    psB = ctx.enter_context(tc.tile_pool(name="psB", bufs=2, space="PSUM"))
    psC = ctx.enter_context(tc.tile_pool(name="psC", bufs=2, space="PSUM"))

    ident = singles.tile([P, P], F32)
    make_identity(nc, ident)
    eps_sb = singles.tile([P, 1], F32)
    nc.vector.memset(eps_sb[:], 1e-5)
    ones8 = singles.tile([1, 8], F16)
    nc.vector.memset(ones8[:], 1.0)

    def evict(dst, src):
        nc.vector.tensor_copy(out=dst, in_=src)

    def t_nat_to_T(src_nat, dst_T, nchunks, npart, uid):
        """(npart, nchunks*128) sbuf -> (128, nchunks, npart) sbuf via PE."""
        for c in range(nchunks):
            ps = psB.tile([P, 8], F32, tag="tp8", name=f"tp_{uid}_{c}")
            nc.tensor.transpose(ps[:, :npart], src_nat[:, c * P:(c + 1) * P],
                                ident[:npart, :npart])
            evict(dst_T[:, c, :], ps[:, :npart])

    def bias_mm(ps, bias_row, n_total, stop=True):
        """Add a (1, n_total) f16 bias row into psum (8, n_total) via ones-row
        matmuls, 512 cols per matmul (moving-dim limit)."""
        nch = (n_total + 511) // 512
        for n in range(nch):
            w = min(512, n_total - n * 512)
            nc.tensor.matmul(ps[:, n * 512:n * 512 + w], ones8[:, :B],
                             bias_row[:, n * 512:n * 512 + w],
                             start=False, stop=(stop and n == nch - 1))

    def layernorm_nat(x_nat, npart, n, y_nat, uid):
        """y = (x - mean) / sqrt(var + eps) over the free dim of (npart, n)."""
        nsub = max(1, n // 512)
        st = nat8.tile([npart, nsub, nc.vector.BN_STATS_DIM], F32, tag="lnst",
                       name=f"lnst_{uid}")
        xg = x_nat.rearrange("p (a b) -> p a b", a=nsub)
        for g in range(nsub):
            nc.vector.bn_stats(out=st[:, g, :], in_=xg[:, g, :])
        mv = nat8.tile([npart, nc.vector.BN_AGGR_DIM], F32, tag="lnmv",
                       name=f"lnmv_{uid}")
        nc.vector.bn_aggr(out=mv[:], in_=st[:])
        std = nat8.tile([npart, 1], F32, tag="lnsd", name=f"lnsd_{uid}")
        nc.scalar.activation(out=std[:], in_=mv[:, 1:2], func=AF.Sqrt,
                             bias=eps_sb[:npart, :])
        nc.vector.reciprocal(out=std[:], in_=std[:])
        nc.vector.tensor_scalar(out=y_nat, in0=x_nat, scalar1=mv[:, 0:1],
                                scalar2=std[:], op0=ALU.subtract, op1=ALU.mult)

    # =======================================================================
    # STEP 0: small constants, bias rows, tail weights — all prefetched
    # early on idle queues so the tail phase never waits on them.
    # =======================================================================
    probe_sb = singles.tile([P, DC], F16)
    nc.sync.dma_start(out=probe_sb[:], in_=t["probe_r"][:])
    bq_sb = singles.tile([1, DH], F32)
    nc.sync.dma_start(out=bq_sb[:], in_=t["bq_s"][:])
    bv_sb = singles.tile([1, DH], F16)
    nc.gpsimd.dma_start(out=bv_sb[:], in_=t["bv16"][:])
    bo_sb = singles.tile([1, D], F16)
    nc.gpsimd.dma_start(out=bo_sb[:], in_=t["bo16"][:])
    b1_sb = singles.tile([1, F1S], F16)
    nc.gpsimd.dma_start(out=b1_sb[:], in_=t["b116"][:])
    b2_sb = singles.tile([1, D], F16)
    nc.gpsimd.dma_start(out=b2_sb[:], in_=t["b216"][:])
    rb_sb = singles.tile([1, HID], F16)
    nc.gpsimd.dma_start(out=rb_sb[:], in_=t["rb16"][:])
    bb1_sb = singles.tile([1, NBLK, 4 * HID], F16)
    nc.gpsimd.dma_start(out=bb1_sb[:], in_=t["blk_b1_16"][:].rearrange("n f -> (n f)")[None, :])
    bb2_sb = singles.tile([1, NBLK, HID], F16)
    nc.gpsimd.dma_start(out=bb2_sb[:], in_=t["blk_b2_16"][:].rearrange("n f -> (n f)")[None, :])
    lng_sb = singles.tile([P, DC], F32)
    nc.sync.dma_start(out=lng_sb[:], in_=t["ln_g_r"][:])
    lnb_sb = singles.tile([P, DC], F32)
    nc.sync.dma_start(out=lnb_sb[:], in_=t["ln_b_r"][:])
    bgr_sb = singles.tile([P, NBLK, HC], F32)
    nc.sync.dma_start(out=bgr_sb[:],
                      in_=t["blk_g_r"][:].rearrange("n p c -> p n c"))
    bbr_sb = singles.tile([P, NBLK, HC], F32)
    nc.sync.dma_start(out=bbr_sb[:],
                      in_=t["blk_b_r"][:].rearrange("n p c -> p n c"))
    rc_sb = singles.tile([TD, HID], F16)
    nc.scalar.dma_start(out=rc_sb[:], in_=t["rin_cond"][:])
    rna_sb = singles.tile([AD, HID], F16)
    nc.scalar.dma_start(out=rna_sb[:], in_=t["rin_na"][:])
    naT_sb = singles.tile([AD, B], F16)
    nc.sync.dma_start(out=naT_sb[:], in_=t["naT"][:])
    ow_sb = singles.tile([P, HC, AD], F16)
    nc.sync.dma_start(out=ow_sb[:],
                      in_=t["out_w"][:].rearrange("(c p) a -> p c a", p=P))
    ob_bc = singles.tile([B, AD], F32)
    nc.gpsimd.dma_start(out=ob_bc[:], in_=_bcast(t["out_bc"][:], B))

    # =======================================================================
    # STEP 1: q = (probe @ wq_s + bq) / sqrt(DH)    -> (1, 512) natural
    # wq is streamed in 8 half-MiB DMAs (4 k-chunks each) on the scalar ring.
    # =======================================================================
    q_nat = singles.tile([1, DH], F32)
    ps_q = psC.tile([1, DH], F32, tag="vec", name="ps_q")
    wq_r = t["wq_s"].rearrange("(c p) n -> p c n", p=P)
    for g in range(8):
        wt = wst.tile([P, 4, DH], F16, tag="wst", name=f"wq_g{g}")
        nc.scalar.dma_start(out=wt[:], in_=wq_r[:, 4 * g:4 * g + 4, :])
        for j in range(4):
            k = 4 * g + j
            nc.tensor.matmul(ps_q[:], probe_sb[:, k:k + 1], wt[:, j, :],
                             start=(k == 0), stop=(k == DC - 1))
    nc.vector.tensor_add(out=q_nat[:], in0=ps_q[:], in1=bq_sb[:])
    nc.vector.tensor_scalar_mul(out=q_nat[:], in0=q_nat[:], scalar1=RSQRT_DH)

    qT = singles.tile([P, DH // P], F16)  # (128, 4)
    for c in range(DH // P):
        ps = psB.tile([P, 8], F32, tag="tp8", name=f"tp_q_{c}")
        nc.tensor.transpose(ps[:, :1], q_nat[:, c * P:(c + 1) * P], ident[:1, :1])
        evict(qT[:, c:c + 1], ps[:, :1])

    # =======================================================================
    # STEP 2: U column of this core's head: U = wkT_s.T @ q~  -> (1, 4096)
    #         AllGather -> cc_u_out (8, 4096) = U.T with one row per head
    # =======================================================================
    u_nat = nat16.tile([1, D], F32, tag="nat16", name="u_nat")
    for nhalf in range(2):
        wk_tiles = []
        for k in range(DH // P):
            wt = wst.tile([P, HD2], F16, tag="wst", name=f"wk_t{nhalf}_{k}")
            nc.scalar.dma_start(
                out=wt[:],
                in_=t["wkT_s"][k * P:(k + 1) * P, nhalf * HD2:(nhalf + 1) * HD2])
            wk_tiles.append(wt)
        for ncol in range(4):
            n0 = nhalf * 4 + ncol
            ps_u = psC.tile([1, DH], F32, tag="vec", name=f"ps_u_{n0}")
            for k in range(DH // P):
                nc.tensor.matmul(
                    ps_u[:], qT[:, k:k + 1],
                    wk_tiles[k][:, ncol * DH:(ncol + 1) * DH],
                    start=(k == 0), stop=(k == DH // P - 1))
            evict(u_nat[:, n0 * DH:(n0 + 1) * DH], ps_u[:])

    nc.gpsimd.dma_start(out=t["cc_u_in"][:], in_=u_nat[:])
    nc.gpsimd.collective_compute(
        "AllGather", ALU.bypass, replica_groups=GROUPS,
        ins=[t["cc_u_in"][:].opt()], outs=[t["cc_u_out"][:].opt()])

    # ---- cond path (fourier + tiny mlp) — independent of everything above,
    # computed here so it is off the critical path of the tail.
    fw_sb = singles.tile([TD, 1], F32)
    nc.sync.dma_start(out=fw_sb[:], in_=t["four_w2"][:])
    ph_sb = singles.tile([TD, 1], F32)
    nc.sync.dma_start(out=ph_sb[:], in_=t["phase2"][:])
    tb32 = singles.tile([TD, B], F32)
    nc.gpsimd.dma_start(out=tb32[:], in_=_bcast(t["timeT"][:], TD))
    fu = singles.tile([TD, B], F32)
    nc.vector.tensor_scalar_mul(out=fu[:], in0=tb32[:], scalar1=fw_sb[:])
    # exact range reduction: sin/cos have period 1 in fu, so subtract the
    # integer part via an f32->i32->f32 round-trip (|fu| < ~64 here).
    fi = singles.tile([TD, B], mybir.dt.int32)
    nc.vector.tensor_copy(out=fi[:], in_=fu[:])
    fif = singles.tile([TD, B], F32)
    nc.vector.tensor_copy(out=fif[:], in_=fi[:])
    nc.vector.tensor_sub(out=fu[:], in0=fu[:], in1=fif[:])
    ffT = singles.tile([TD, B], F16)
    nc.scalar.activation(out=ffT[:], in_=fu[:], func=AF.Sin,
                         scale=TWO_PI, bias=ph_sb[:])
    cw1_sb = singles.tile([TD, 2 * TD], F16)
    nc.scalar.dma_start(out=cw1_sb[:], in_=t["cond_w1"][:])
    cb1_sb = singles.tile([2 * TD, 1], F32)
    nc.sync.dma_start(out=cb1_sb[:], in_=t["cond_b1c"][:])
    cw2_sb = singles.tile([2 * TD, TD], F16)
    nc.scalar.dma_start(out=cw2_sb[:], in_=t["cond_w2"][:])
    cb2_sb = singles.tile([TD, 1], F32)
    nc.sync.dma_start(out=cb2_sb[:], in_=t["cond_b2c"][:])
    ps_c1 = psB.tile([P, 8], F32, tag="tp8", name="ps_c1")
    nc.tensor.matmul(ps_c1[:2 * TD, :B], cw1_sb[:], ffT[:], start=True, stop=True)
    c1 = singles.tile([2 * TD, B], F16)
    nc.scalar.activation(out=c1[:], in_=ps_c1[:2 * TD, :B], func=AF.Silu,
                         bias=cb1_sb[:])
    ps_c2 = psB.tile([P, 8], F32, tag="tp8", name="ps_c2")
    nc.tensor.matmul(ps_c2[:TD, :B], cw2_sb[:], c1[:], start=True, stop=True)
    condT = singles.tile([TD, B], F16)
    nc.scalar.activation(out=condT[:], in_=ps_c2[:TD, :B], func=AF.Identity,
                         bias=cb2_sb[:])

    # ---- read back U.T (8, 4096), transpose to (128, 32, 8), cast to f16
    uh_nat = nat16.tile([H, D], F32, tag="nat16", name="uh_nat")
    nc.sync.dma_start(out=uh_nat[:], in_=t["cc_u_out"][:])
    u_f16 = singles.tile([P, DC, H], F16)
    for c in range(DC):
        ps = psB.tile([P, 8], F32, tag="tp8", name=f"tp_u_{c}")
        nc.tensor.transpose(ps[:, :H], uh_nat[:, c * P:(c + 1) * P],
                            ident[:H, :H])
        evict(u_f16[:, c, :], ps[:, :H])

    # =======================================================================
    # STEP 3: scoresT (8, 2048) = U.T @ llmT  (fp16 inputs, fp32 accum)
    # =======================================================================
    ps_sc = psA.tile([H, S], F32, tag="big", name="ps_sc")
    for k in range(DC):
        lt = llmT_pool.tile([P, S], F16, tag="llmT", name=f"llmT_t{k}")
        nc.sync.dma_start(out=lt[:], in_=t["llmT"][k * P:(k + 1) * P, :])
        for n in range(S // 512):
            nc.tensor.matmul(ps_sc[:, n * 512:(n + 1) * 512],
                             u_f16[:, k, :], lt[:, n * 512:(n + 1) * 512],
                             start=(k == 0), stop=(k == DC - 1))

    # =======================================================================
    # STEP 4: softmax over S. Max-subtraction is skipped deliberately:
    # softmax is shift-invariant and |scores| here is < ~1, so exp() is
    # perfectly conditioned; the result is mathematically identical.
    # =======================================================================
    p_nat = nat8.tile([H, S], F32, tag="nat8", name="p_nat")
    nc.scalar.activation(out=p_nat[:], in_=ps_sc[:], func=AF.Exp)
    den = singles.tile([H, 1], F32)
    nc.vector.reduce_sum(out=den[:], in_=p_nat[:], axis=mybir.AxisListType.X)
    nc.vector.reciprocal(out=den[:], in_=den[:])
    nc.vector.tensor_scalar_mul(out=p_nat[:], in0=p_nat[:], scalar1=den[:])
    pT = singles.tile([P, SC, H], F16)
    t_nat_to_T(p_nat, pT, SC, H, "p")

    # =======================================================================
    # STEP 5: pooled (8, 4096) = pT.T @ llm ; AllToAll (head <-> batch)
    # =======================================================================
    pooled_nat = nat16.tile([H, D], F32, tag="nat16", name="pooled_nat")
    for half in range(2):
        ps_p = psA.tile([H, HD2], F32, tag="big", name=f"ps_pool_{half}")
        for s in range(SC):
            lt = llm_pool.tile([P, HD2], F16, tag="llm", name=f"llm_t{half}_{s}")
            nc.sync.dma_start(
                out=lt[:],
                in_=t["llm"][s * P:(s + 1) * P, half * HD2:(half + 1) * HD2])
            for n in range(HD2 // 512):
                nc.tensor.matmul(ps_p[:, n * 512:(n + 1) * 512],
                                 pT[:, s, :], lt[:, n * 512:(n + 1) * 512],
                                 start=(s == 0), stop=(s == SC - 1))
        evict(pooled_nat[:, half * HD2:(half + 1) * HD2], ps_p[:])

    nc.gpsimd.dma_start(out=t["cc_pool_in"][:], in_=pooled_nat[:])
    nc.gpsimd.collective_compute(
        "AllToAll", ALU.bypass, replica_groups=GROUPS,
        ins=[t["cc_pool_in"][:].opt()], outs=[t["cc_pool_out"][:].opt()])

    # =======================================================================
    # STEP 6: ctx for this core's head, all batches: (8, 512) = poolh@wv + bv
    # =======================================================================
    poolh_nat = nat16.tile([B, D], F32, tag="nat16", name="poolh_nat")
    nc.sync.dma_start(out=poolh_nat[:], in_=t["cc_pool_out"][:])
    poolhT = singles.tile([P, DC, B], F16)
    t_nat_to_T(poolh_nat, poolhT, DC, B, "ph")

    ps_cx = psA.tile([B, DH], F32, tag="big", name="ps_cx")
    wv_r = t["wv_s"].rearrange("(c p) n -> p c n", p=P)
    for g in range(8):
        wt = wst.tile([P, 4, DH], F16, tag="wst", name=f"wv_g{g}")
        nc.scalar.dma_start(out=wt[:], in_=wv_r[:, 4 * g:4 * g + 4, :])
        for j in range(4):
            k = 4 * g + j
            nc.tensor.matmul(ps_cx[:], poolhT[:, k, :], wt[:, j, :],
                             start=(k == 0), stop=False)
    bias_mm(ps_cx, bv_sb, DH)
    ctx_nat = nat8.tile([B, DH], F32, tag="nat8", name="ctx_nat")
    evict(ctx_nat[:], ps_cx[:])
    ctxT = singles.tile([P, DH // P, B], F16)
    t_nat_to_T(ctx_nat, ctxT, DH // P, B, "cx")

    # =======================================================================
    # STEP 7: attn partial (8, 4096) = ctx @ wo_s + bo/8 ; AllReduce
    # =======================================================================
    attn_part = nat16.tile([B, D], F32, tag="nat16", name="attn_part")
    for half in range(2):
        ps_a = psA.tile([B, HD2], F32, tag="big", name=f"ps_attn_{half}")
        for k in range(DH // P):
            wt = wst.tile([P, HD2], F16, tag="wst", name=f"wo_t{half}_{k}")
            nc.scalar.dma_start(
                out=wt[:],
                in_=t["wo_s"][k * P:(k + 1) * P, half * HD2:(half + 1) * HD2])
            for n in range(HD2 // 512):
                nc.tensor.matmul(ps_a[:, n * 512:(n + 1) * 512],
                                 ctxT[:, k, :], wt[:, n * 512:(n + 1) * 512],
                                 start=(k == 0), stop=False)
        bias_mm(ps_a, bo_sb[:, half * HD2:(half + 1) * HD2], HD2)
        evict(attn_part[:, half * HD2:(half + 1) * HD2], ps_a[:])
    nc.gpsimd.dma_start(out=t["cc_attn_in"][:], in_=attn_part[:])
    nc.gpsimd.collective_compute(
        "AllReduce", ALU.add, replica_groups=GROUPS,
        ins=[t["cc_attn_in"][:].opt()], outs=[t["cc_attn_out"][:].opt()])

    # =======================================================================
    # STEP 8: y = LN(attn_out)*g+b ; mlp partial (+b1, gelu, @w2 + b2/8) ; AR
    # =======================================================================
    attn_nat = singles.tile([B, D], F32)  # persists (residual)
    nc.sync.dma_start(out=attn_nat[:], in_=t["cc_attn_out"][:])

    y_nat = nat16.tile([B, D], F32, tag="nat16", name="y_nat")
    layernorm_nat(attn_nat[:], B, D, y_nat[:], "ln0")
    yT = singles.tile([P, DC, B], F16)
    t_nat_to_T(y_nat, yT, DC, B, "y")
    # LN affine in T layout (gamma/beta become per-partition scalars)
    for c in range(DC):
        nc.vector.tensor_scalar(out=yT[:, c, :], in0=yT[:, c, :],
                                scalar1=lng_sb[:, c:c + 1],
                                scalar2=lnb_sb[:, c:c + 1],
                                op0=ALU.mult, op1=ALU.add)

    # mm1: h1 (8, 2048) = y @ w1_s + b1 ; exact gelu straight off PSUM
    ps_h1 = psA.tile([B, F1S], F32, tag="big", name="ps_h1")
    for k in range(DC):
        wt = wst.tile([P, F1S], F16, tag="wst", name=f"w1_t{k}")
        nc.scalar.dma_start(out=wt[:], in_=t["w1_s"][k * P:(k + 1) * P, :])
        for n in range(F1S // 512):
            nc.tensor.matmul(ps_h1[:, n * 512:(n + 1) * 512],
                             yT[:, k, :], wt[:, n * 512:(n + 1) * 512],
                             start=(k == 0), stop=False)
    bias_mm(ps_h1, b1_sb, F1S)
    g_nat = nat8.tile([B, F1S], F32, tag="nat8", name="g_nat")
    nc.scalar.activation(out=g_nat[:], in_=ps_h1[:], func=AF.Gelu)
    gT = singles.tile([P, F1S // P, B], F16)
    t_nat_to_T(g_nat, gT, F1S // P, B, "g")

    # mm2: h2 partial (8, 4096) = g @ w2_s + b2/8 ; AllReduce
    h2_nat = nat16.tile([B, D], F32, tag="nat16", name="h2_nat")
    for half in range(2):
        ps_h2 = psA.tile([B, HD2], F32, tag="big", name=f"ps_h2_{half}")
        for k in range(F1S // P):
            wt = wst.tile([P, HD2], F16, tag="wst", name=f"w2_t{half}_{k}")
            nc.scalar.dma_start(
                out=wt[:],
                in_=t["w2_s"][k * P:(k + 1) * P, half * HD2:(half + 1) * HD2])
            for n in range(HD2 // 512):
                nc.tensor.matmul(ps_h2[:, n * 512:(n + 1) * 512],
                                 gT[:, k, :], wt[:, n * 512:(n + 1) * 512],
                                 start=(k == 0), stop=False)
        bias_mm(ps_h2, b2_sb[:, half * HD2:(half + 1) * HD2], HD2)
        evict(h2_nat[:, half * HD2:(half + 1) * HD2], ps_h2[:])
    nc.gpsimd.dma_start(out=t["cc_mlp_in"][:], in_=h2_nat[:])
    nc.gpsimd.collective_compute(
        "AllReduce", ALU.add, replica_groups=GROUPS,
        ins=[t["cc_mlp_in"][:].opt()], outs=[t["cc_mlp_out"][:].opt()])

    # =======================================================================
    # STEP 9: x_pool = attn_out + h ; diffusion tail (replicated on all cores)
    # =======================================================================
    hug = nat16.tile([B, D], F32, tag="nat16", name="hug")
    nc.sync.dma_start(out=hug[:], in_=t["cc_mlp_out"][:])
    nc.vector.tensor_add(out=attn_nat[:], in0=attn_nat[:], in1=hug[:])
    xpT = singles.tile([P, DC, B], F16)
    t_nat_to_T(attn_nat, xpT, DC, B, "xp")

    # x0 (8, 256) = x_pool@rin_pool + cond@rin_cond + na@rin_na + rin_b
    ps_x0 = psA.tile([B, HID], F32, tag="big", name="ps_x0")
    for k in range(DC):
        wt = wst.tile([P, HID], F16, tag="wst", name=f"rp_t{k}")
        nc.scalar.dma_start(out=wt[:], in_=t["rin_pool"][k * P:(k + 1) * P, :])
        nc.tensor.matmul(ps_x0[:], xpT[:, k, :], wt[:], start=(k == 0),
                         stop=False)
    nc.tensor.matmul(ps_x0[:], condT[:], rc_sb[:], start=False, stop=False)
    nc.tensor.matmul(ps_x0[:], naT_sb[:], rna_sb[:], start=False, stop=False)
    bias_mm(ps_x0, rb_sb, HID)
    x_nat = singles.tile([B, HID], F32)
    evict(x_nat[:], ps_x0[:])

    # ---- 3 residual blocks ----
    for i in range(NBLK):
        xn = singles.tile([B, HID], F32, name=f"xn_{i}")
        layernorm_nat(x_nat[:], B, HID, xn[:], f"lnb{i}")
        xnT = singles.tile([P, HC, B], F16, name=f"xnT_{i}")
        t_nat_to_T(xn, xnT, HC, B, f"xn{i}")
        for c in range(HC):  # LN affine in T layout
            nc.vector.tensor_scalar(out=xnT[:, c, :], in0=xnT[:, c, :],
                                    scalar1=bgr_sb[:, i, c:c + 1],
                                    scalar2=bbr_sb[:, i, c:c + 1],
                                    op0=ALU.mult, op1=ALU.add)

        ps_bh = psA.tile([B, 4 * HID], F32, tag="big", name=f"ps_bh_{i}")
        for k in range(HC):
            wt = wst.tile([P, 4 * HID], F16, tag="wst", name=f"bw1_t{i}_{k}")
            nc.scalar.dma_start(out=wt[:], in_=t["blk_w1"][i, k * P:(k + 1) * P, :])
            for n in range(4 * HID // 512):
                nc.tensor.matmul(ps_bh[:, n * 512:(n + 1) * 512],
                                 xnT[:, k, :], wt[:, n * 512:(n + 1) * 512],
                                 start=(k == 0), stop=False)
        bias_mm(ps_bh, bb1_sb[:, i, :], 4 * HID)
        hb = nat8.tile([B, 4 * HID], F32, tag="nat8", name=f"hb_{i}")
        nc.scalar.activation(out=hb[:], in_=ps_bh[:], func=AF.Silu)
        hbT = singles.tile([P, 4 * HID // P, B], F16, name=f"hbT_{i}")
        t_nat_to_T(hb, hbT, 4 * HID // P, B, f"hb{i}")

        ps_bo = psA.tile([B, HID], F32, tag="big", name=f"ps_bo_{i}")
        for k in range(4 * HID // P):
            wt = wst.tile([P, HID], F16, tag="wst", name=f"bw2_t{i}_{k}")
            nc.scalar.dma_start(out=wt[:], in_=t["blk_w2"][i, k * P:(k + 1) * P, :])
            nc.tensor.matmul(ps_bo[:], hbT[:, k, :], wt[:],
                             start=(k == 0), stop=False)
        bias_mm(ps_bo, bb2_sb[:, i, :], HID)
        nc.vector.tensor_add(out=x_nat[:], in0=x_nat[:], in1=ps_bo[:])

    # ---- final: res (8, 7) = swish(x) @ out_w + out_b
    nc.scalar.activation(out=x_nat[:], in_=x_nat[:], func=AF.Silu)
    xsT = singles.tile([P, HC, B], F16)
    t_nat_to_T(x_nat, xsT, HC, B, "xs")
    ps_o = psB.tile([P, 8], F32, tag="tp8", name="ps_o")
    for k in range(HC):
        nc.tensor.matmul(ps_o[:B, :AD], xsT[:, k, :], ow_sb[:, k, :],
                         start=(k == 0), stop=(k == HC - 1))
    out_sb = singles.tile([B, AD], F32)
    nc.vector.tensor_add(out=out_sb[:], in0=ps_o[:B, :AD], in1=ob_bc[:])
    nc.sync.dma_start(out=t["res"][:], in_=out_sb[:])


_CACHED_NC = None


def _get_nc():
    global _CACHED_NC
    if _CACHED_NC is None:
        _CACHED_NC = build_program()
    return _CACHED_NC


def _prep_in_maps(inputs):
    f32 = np.float32
    f16 = np.float16
    llm_full = np.ascontiguousarray(np.asarray(inputs["llm_output"], dtype=f32))
    wq = np.asarray(inputs["wq"], f32); wk = np.asarray(inputs["wk"], f32)
    wv = np.asarray(inputs["wv"], f32); wo = np.asarray(inputs["wo"], f32)
    bq = np.asarray(inputs["bq"], f32); bv = np.asarray(inputs["bv"], f32)
    bo = np.asarray(inputs["bo"], f32)
    w1 = np.asarray(inputs["mlp_w1"], f32); b1 = np.asarray(inputs["mlp_b1"], f32)
    w2 = np.asarray(inputs["mlp_w2"], f32); b2 = np.asarray(inputs["mlp_b2"], f32)
    rin_w = np.asarray(inputs["rin_w"], f32)
    probe = np.asarray(inputs["probe"], f32).reshape(D)

    def r128(v):  # (n*128,) -> (128, n) partition-major
        return np.ascontiguousarray(v.reshape(-1, P).T)

    blk_g = np.asarray(inputs["blk_ln_g"], f32)
    blk_b = np.asarray(inputs["blk_ln_b"], f32)

    shared = {
        "bo16": (bo / NC).astype(f16).reshape(1, D),
        "ln_g_r": r128(np.asarray(inputs["ln_g"], f32)),
        "ln_b_r": r128(np.asarray(inputs["ln_b"], f32)),
        "b216": (b2 / NC).astype(f16).reshape(1, D),
        "probe_r": r128(probe).astype(f16),
        "four_w2": np.concatenate(
            [np.asarray(inputs["four_w"], f32).reshape(TD // 2, 1)] * 2),
        "phase2": np.concatenate(
            [np.full((TD // 2, 1), np.pi / 2, f32),
             np.zeros((TD // 2, 1), f32)]),
        "timeT": np.ascontiguousarray(np.asarray(inputs["time"], f32).T),
        "naT": np.ascontiguousarray(
            np.asarray(inputs["noisy_actions"], f32).T).astype(f16),
        "cond_w1": np.asarray(inputs["cond_w1"], f32).astype(f16),
        "cond_b1c": np.asarray(inputs["cond_b1"], f32).reshape(-1, 1),
        "cond_w2": np.asarray(inputs["cond_w2"], f32).astype(f16),
        "cond_b2c": np.asarray(inputs["cond_b2"], f32).reshape(-1, 1),
        "rin_cond": np.ascontiguousarray(rin_w[0:TD]).astype(f16),
        "rin_pool": np.ascontiguousarray(rin_w[TD:TD + D]).astype(f16),
        "rin_na": np.ascontiguousarray(rin_w[TD + D:]).astype(f16),
        "rb16": np.asarray(inputs["rin_b"], f32).astype(f16).reshape(1, HID),
        "blk_g_r": np.ascontiguousarray(
            blk_g.reshape(NBLK, HC, P).transpose(0, 2, 1)),
        "blk_b_r": np.ascontiguousarray(
            blk_b.reshape(NBLK, HC, P).transpose(0, 2, 1)),
        "blk_w1": np.asarray(inputs["blk_w1"], f32).astype(f16),
        "blk_b1_16": np.asarray(inputs["blk_b1"], f32).astype(f16),
        "blk_w2": np.asarray(inputs["blk_w2"], f32).astype(f16),
        "blk_b2_16": np.asarray(inputs["blk_b2"], f32).astype(f16),
        "out_w": np.asarray(inputs["out_w"], f32).astype(f16),
        "out_bc": np.asarray(inputs["out_b"], f32).reshape(1, AD),
    }

    in_maps = []
    for i in range(NC):
        hb = slice(i * DH, (i + 1) * DH)
        fb = slice(i * F1S, (i + 1) * F1S)
        m = dict(shared)
        m["llm"] = llm_full[i].astype(f16)
        m["llmT"] = np.ascontiguousarray(llm_full[i].T).astype(f16)
        m["wq_s"] = np.ascontiguousarray(wq[:, hb]).astype(f16)
        m["bq_s"] = np.ascontiguousarray(bq[hb]).reshape(1, DH)
        m["wkT_s"] = np.ascontiguousarray(wk[:, hb].T).astype(f16)
        m["wv_s"] = np.ascontiguousarray(wv[:, hb]).astype(f16)
        m["bv16"] = np.ascontiguousarray(bv[hb]).astype(f16).reshape(1, DH)
        m["wo_s"] = np.ascontiguousarray(wo[hb, :]).astype(f16)
        m["w1_s"] = np.ascontiguousarray(w1[:, fb]).astype(f16)
        m["b116"] = np.ascontiguousarray(b1[fb]).astype(f16).reshape(1, F1S)
        m["w2_s"] = np.ascontiguousarray(w2[fb, :]).astype(f16)
        in_maps.append(m)
    return in_maps


def kernel(**inputs):
    nc = _get_nc()
    in_maps = _prep_in_maps(inputs)
    r = run_bass_kernel_spmd(nc, in_maps, core_ids=list(range(NC)))
    return np.ascontiguousarray(r.results[0]["res"]).astype(np.float32)


def run_traced(**inputs):
    """Like kernel() but with NTFF tracing; returns (output, results)."""
    nc = _get_nc()
    in_maps = _prep_in_maps(inputs)
    r = run_bass_kernel_spmd(nc, in_maps, core_ids=list(range(NC)), trace=True)
    return np.ascontiguousarray(r.results[0]["res"]).astype(np.float32), r



# revision 7
# speedup vs baseline: 1.3675x; 1.3675x over previous
"""Trainium2 Bass kernel for nn_DiffusionActionHead (B=8, S=2048, D=4096).

v2 strategy (8 NeuronCores, batch-parallel + head-parallel):
  - Host folds weight-only math:  U = wk^T (probe@wq + bq) / sqrt(DH)
    (removes the wq/wk streams and the U AllGather), and
    w2rin = mlp_w2 @ rin_w[pool rows]:  (attn_out + h) is consumed ONLY
    through rin_w, so the 16 MiB w2 stream collapses to 1 MiB and the mlp
    AllReduce shrinks from 128 KiB to the 8 KiB x0 AllReduce.
  - Scores stream llm^T in fp8 (e3m4, 8 MiB): softmax washes the ~4%
    quantization noise down to ~0.2% on attention weights.  The pooled
    pass streams llm natural in f16 (fp8 would put ~3% on the output).
  - All m=8 matmuls are 4-way column-tiled (tile_position): measured
    2.35x PE throughput on this part.  Wide outputs use tiles-over-n
    (no cross-tile fixup); narrow outputs use tiles-over-k + 3 DVE adds.
  - Biases enter PSUM via 128-row replicated bias tiles (ones/128
    stationary) so every matmul keeps the same (128,32) PE tiling mode.
  - DMA: 0.5-2 MiB mostly-contiguous transfers.  scalar ring: llm
    streams; sync ring: rin_pool + w1 (deep window) + tail weights;
    gpsimd: smalls, wv, wo, collective bounces (with f16<->f32 casts).
  - Collectives: AllToAll(pooled, f16 wire), AllReduce(attn, f32 wire),
    AllReduce(x0 partials, 8 KiB).  Diffusion tail replicated.
"""

import numpy as np
import sys

if "/opt/trn_rl_repo" not in sys.path:
    sys.path.insert(0, "/opt/trn_rl_repo")

import ml_dtypes
import concourse.bass as bass
import concourse.tile as tile
from concourse import bacc, mybir
from concourse.masks import make_identity
from concourse.bass_utils import run_bass_kernel_spmd

F32 = mybir.dt.float32
F16 = mybir.dt.float16
F8 = mybir.dt.float8e3
NP8 = ml_dtypes.float8_e3m4
AF = mybir.ActivationFunctionType
ALU = mybir.AluOpType

B, S, D = 8, 2048, 4096
H, AD, TD, HID, NBLK = 8, 7, 32, 256, 3
DH = D // H            # 512
NC = 8
P = 128
SC = S // P            # 16
DC = D // P            # 32
F1S = 4 * D // NC      # 2048
FC = F1S // P          # 16
HC = HID // P          # 2
SU = 2048.0            # fp8 scale on U (exp() undoes it)
RSQRT_DH = 1.0 / float(np.sqrt(DH))
TWO_PI = 2.0 * float(np.pi)

# brep column offsets (replicated bias rows, f16)
BO_O = 0                  # bo/8   (4096)
B1_O = 4096               # b1     (2048)
BV_O = 6144               # bv     (512)
BB1_O = 6656              # blk_b1 (3*1024)
BB2_O = 9728              # blk_b2 (3*256)
BREP_N = 10496


def _bcast(src_ap, nparts):
    ap = src_ap
    assert ap.shape[0] == 1, ap.shape
    return bass.AP(tensor=ap.tensor, offset=ap.offset,
                   ap=[[0, nparts]] + [list(x) for x in ap.ap[1:]])


def build_program():
    nc = bacc.Bacc("TRN2", target_bir_lowering=False, debug=False,
                   num_devices=NC)
    t = {}

    def din(name, shape, dtype=F32):
        t[name] = nc.dram_tensor(name, shape, dtype, kind="ExternalInput")

    din("llm16", [S, D], F16)
    din("llmT8", [D, S], F8)
    din("U8r", [P, DC, H], F8)
    din("wv16", [8, P, 4, DH], F16)
    din("wo16", [DH, D], F16)
    din("w116", [16, P, 2, F1S], F16)
    din("w2rin16", [P, FC, HID], F16)
    din("rp16", [P, DC, HID], F16)
    din("brep", [1, BREP_N], F16)
    din("ln_g_r", [P, DC]); din("ln_b_r", [P, DC])
    din("four_w2", [TD, 1]); din("phase2", [TD, 1])
    din("timeT", [1, B]); din("naT", [AD, B], F16)
    din("cw1", [TD, 2 * TD], F16); din("cb1c", [2 * TD, 1])
    din("cw2rin8", [2 * TD, HID], F16)
    din("rna8", [AD, HID], F16)
    din("rb8", [1, HID], F16)
    din("bgr", [P, NBLK, HC]); din("bbr", [P, NBLK, HC])
    din("bw1a", [P, 3, 4 * HID], F16)
    din("bw1b", [P, 3, 4 * HID], F16)
    din("bw2a", [P, 12, HID], F16)
    din("bw2b", [P, 12, HID], F16)
    din("ow", [P, HC, AD], F16); din("out_bc", [1, AD])
    t["res"] = nc.dram_tensor("res", [B, AD], F32, kind="ExternalOutput")

    t["cc_pool_in"] = nc.dram_tensor("cc_pool_in", [H, D], F16)
    t["cc_pool_out"] = nc.dram_tensor("cc_pool_out", [B, D], F16)
    t["cc_attn_in"] = nc.dram_tensor("cc_attn_in", [B, D], F32)
    t["cc_attn_out"] = nc.dram_tensor("cc_attn_out", [B, D], F32,
                                      addr_space="Shared")
    t["cc_x0_in"] = nc.dram_tensor("cc_x0_in", [B, HID], F32)
    t["cc_x0_out"] = nc.dram_tensor("cc_x0_out", [B, HID], F32,
                                    addr_space="Shared")

    with tile.TileContext(nc) as tc:
        import contextlib
        with contextlib.ExitStack() as ctx:
            _build(nc, tc, t, ctx)
    nc.finalize()
    return nc


def _build(nc, tc, t, ctx):
    GROUPS = [list(range(NC))]

    singles = ctx.enter_context(tc.tile_pool(name="singles", bufs=1))
    lt8p = ctx.enter_context(tc.tile_pool(name="lt8p", bufs=2))
    ln16p = ctx.enter_context(tc.tile_pool(name="ln16p", bufs=2))
    natp = ctx.enter_context(tc.tile_pool(name="natp", bufs=2))
    wvp = ctx.enter_context(tc.tile_pool(name="wvp", bufs=8))
    wop = ctx.enter_context(tc.tile_pool(name="wop", bufs=2))
    w1p = ctx.enter_context(tc.tile_pool(name="w1p", bufs=5))
    psA = ctx.enter_context(tc.tile_pool(name="psA", bufs=2, space="PSUM"))
    psB = ctx.enter_context(tc.tile_pool(name="psB", bufs=2, space="PSUM"))
    psT8 = ctx.enter_context(tc.tile_pool(name="psT8", bufs=2, space="PSUM"))

    ident = singles.tile([P, P], F32)
    make_identity(nc, ident)
    ident16 = singles.tile([P, P], F16)
    nc.vector.tensor_copy(out=ident16[:], in_=ident[:])
    eps_sb = singles.tile([P, 1], F32)
    nc.vector.memset(eps_sb[:], 1e-5)
    ones8 = singles.tile([1, 8], F16)
    nc.vector.memset(ones8[:], 1.0)
    ones128 = singles.tile([P, 8], F16)
    nc.vector.memset(ones128[:], 1.0 / 128.0)

    def t_nat_to_T(src_nat, dst_T, nchunks, npart, uid, evict_eng=None):
        """f16 (npart, nchunks*128) -> (128, nchunks, npart) via PE."""
        eng = evict_eng or nc.vector
        for c in range(nchunks):
            ps = psT8.tile([P, 16], F16, tag="tp16", name=f"tp_{uid}_{c}")
            nc.tensor.transpose(ps[:, :npart], src_nat[:, c * P:(c + 1) * P],
                                ident16[:npart, :npart])
            if eng is nc.scalar:
                nc.scalar.activation(out=dst_T[:, c, :], in_=ps[:, :npart],
                                     func=AF.Identity)
            else:
                eng.tensor_copy(out=dst_T[:, c, :], in_=ps[:, :npart])

    def bias_rep(ps_slice, col0, n_total, tp, stop):
        """Add brep[col0:col0+n] into a psum quadrant via ones/128 matmul."""
        nchn = (n_total + 511) // 512
        for n in range(nchn):
            w = min(512, n_total - n * 512)
            nc.tensor.matmul(
                ps_slice[:, n * 512:n * 512 + w], ones128[:, :B],
                brep_sb[:, col0 + n * 512:col0 + n * 512 + w],
                start=False, stop=stop, tile_position=tp)

    def quad_sum(dst, ps, n):
        nc.vector.tensor_copy(out=dst, in_=ps[0:B, :n])
        nc.vector.tensor_add(out=dst, in0=dst, in1=ps[32:32 + B, :n])
        nc.vector.tensor_add(out=dst, in0=dst, in1=ps[64:64 + B, :n])
        nc.vector.tensor_add(out=dst, in0=dst, in1=ps[96:96 + B, :n])

    def layernorm_nat(x_nat, npart, n, y_nat, uid):
        nsub = max(1, n // 512)
        st = singles.tile([npart, nsub, nc.vector.BN_STATS_DIM], F32,
                          name=f"lnst_{uid}")
        xg = x_nat.rearrange("p (a b) -> p a b", a=nsub)
        for g in range(nsub):
            nc.vector.bn_stats(out=st[:, g, :], in_=xg[:, g, :])
        mv = singles.tile([npart, nc.vector.BN_AGGR_DIM], F32,
                          name=f"lnmv_{uid}")
        nc.vector.bn_aggr(out=mv[:], in_=st[:])
        std = singles.tile([npart, 1], F32, name=f"lnsd_{uid}")
        nc.scalar.activation(out=std[:], in_=mv[:, 1:2], func=AF.Sqrt,
                             bias=eps_sb[:npart, :])
        nc.vector.reciprocal(out=std[:], in_=std[:])
        nc.vector.tensor_scalar(out=y_nat, in0=x_nat, scalar1=mv[:, 0:1],
                                scalar2=std[:], op0=ALU.subtract, op1=ALU.mult)

    # =======================================================================
    # STEP 0: smalls on gpsimd; U + rin_pool on sync.
    # =======================================================================
    u8_sb = singles.tile([P, DC, H], F8)
    nc.sync.dma_start(out=u8_sb[:], in_=t["U8r"][:])
    rp_sb = singles.tile([P, DC, HID], F16)
    nc.sync.dma_start(out=rp_sb[:], in_=t["rp16"][:])

    brep_sb = singles.tile([P, BREP_N], F16)
    nc.gpsimd.dma_start(out=brep_sb[:], in_=_bcast(t["brep"][:], P))
    rb_sb = singles.tile([1, HID], F16)
    nc.gpsimd.dma_start(out=rb_sb[:], in_=t["rb8"][:])
    lng_sb = singles.tile([P, DC], F32)
    nc.gpsimd.dma_start(out=lng_sb[:], in_=t["ln_g_r"][:])
    lnb_sb = singles.tile([P, DC], F32)
    nc.gpsimd.dma_start(out=lnb_sb[:], in_=t["ln_b_r"][:])
    bgr_sb = singles.tile([P, NBLK, HC], F32)
    nc.gpsimd.dma_start(out=bgr_sb[:], in_=t["bgr"][:])
    bbr_sb = singles.tile([P, NBLK, HC], F32)
    nc.gpsimd.dma_start(out=bbr_sb[:], in_=t["bbr"][:])
    naT_sb = singles.tile([AD, B], F16)
    nc.gpsimd.dma_start(out=naT_sb[:], in_=t["naT"][:])
    rna_sb = singles.tile([AD, HID], F16)
    nc.gpsimd.dma_start(out=rna_sb[:], in_=t["rna8"][:])
    cw1_sb = singles.tile([TD, 2 * TD], F16)
    nc.gpsimd.dma_start(out=cw1_sb[:], in_=t["cw1"][:])
    cb1_sb = singles.tile([2 * TD, 1], F32)
    nc.gpsimd.dma_start(out=cb1_sb[:], in_=t["cb1c"][:])
    cwr_sb = singles.tile([2 * TD, HID], F16)
    nc.gpsimd.dma_start(out=cwr_sb[:], in_=t["cw2rin8"][:])
    fw_sb = singles.tile([TD, 1], F32)
    nc.gpsimd.dma_start(out=fw_sb[:], in_=t["four_w2"][:])
    ph_sb = singles.tile([TD, 1], F32)
    nc.gpsimd.dma_start(out=ph_sb[:], in_=t["phase2"][:])
    tb32 = singles.tile([TD, B], F32)
    nc.gpsimd.dma_start(out=tb32[:], in_=_bcast(t["timeT"][:], TD))
    ow_sb = singles.tile([P, HC, AD], F16)
    nc.gpsimd.dma_start(out=ow_sb[:], in_=t["ow"][:])
    ob_bc = singles.tile([B, AD], F32)
    nc.gpsimd.dma_start(out=ob_bc[:], in_=_bcast(t["out_bc"][:], B))
    # wv resident early (needed right after the A2A with no ring slack)
    wv_tiles = []
    for g in range(8):
        wt = wvp.tile([P, 4, DH], F16, tag="wv", name=f"wv_{g}")
        nc.gpsimd.dma_start(out=wt[:], in_=t["wv16"][g])
        wv_tiles.append(wt)

    # =======================================================================
    # STEP 1: cond path + x0_early = c1^T@cw2rin/8 + na@rna/8 + rb/8
    # =======================================================================
    fu = singles.tile([TD, B], F32)
    nc.vector.tensor_scalar_mul(out=fu[:], in0=tb32[:], scalar1=fw_sb[:])
    fi = singles.tile([TD, B], mybir.dt.int32)
    nc.vector.tensor_copy(out=fi[:], in_=fu[:])
    fif = singles.tile([TD, B], F32)
    nc.vector.tensor_copy(out=fif[:], in_=fi[:])
    nc.vector.tensor_sub(out=fu[:], in0=fu[:], in1=fif[:])
    ffT = singles.tile([TD, B], F16)
    nc.scalar.activation(out=ffT[:], in_=fu[:], func=AF.Sin,
                         scale=TWO_PI, bias=ph_sb[:])
    ps_c1 = psB.tile([P, 512], F32, tag="psB", name="ps_c1")
    nc.tensor.matmul(ps_c1[:2 * TD, :B], cw1_sb[:], ffT[:], start=True,
                     stop=True)
    c1 = singles.tile([2 * TD, B], F16)
    nc.scalar.activation(out=c1[:], in_=ps_c1[:2 * TD, :B], func=AF.Silu,
                         bias=cb1_sb[:])
    ps_e = psB.tile([P, 512], F32, tag="psB", name="ps_e")
    nc.tensor.matmul(ps_e[:B, :HID], c1[:], cwr_sb[:], start=True, stop=False)
    nc.tensor.matmul(ps_e[:B, :HID], naT_sb[:], rna_sb[:], start=False,
                     stop=False)
    nc.tensor.matmul(ps_e[:B, :HID], ones8[:, :B], rb_sb[:], start=False,
                     stop=True)
    x0_early = singles.tile([B, HID], F32)
    nc.vector.tensor_copy(out=x0_early[:], in_=ps_e[:B, :HID])

    # =======================================================================
    # STEP 2: scoresT (8h, 2048s) = (U*SU)^T @ llmT  [fp8, tiles-over-n]
    # =======================================================================
    ps_sc = psA.tile([P, 1024], F32, tag="psA", name="ps_sc")
    for g in range(8):
        lt = lt8p.tile([P, 4, S], F8, tag="lt8", name=f"lt8_{g}")
        nc.scalar.dma_start(
            out=lt[:],
            in_=t["llmT8"][g * 512:(g + 1) * 512, :].rearrange(
                "(c p) s -> p c s", p=P))
        for cc in range(4):
            c = 4 * g + cc
            for j in range(4):
                nc.tensor.matmul(
                    ps_sc[32 * j:32 * j + H, 0:512],
                    u8_sb[:, c, :], lt[:, cc, 512 * j:512 * (j + 1)],
                    start=(c == 0), stop=(c == DC - 1),
                    tile_position=(0, 32 * j))

    p_nat = natp.tile([H, S], F16, tag="nat8", name="p_nat")
    for j in range(4):
        nc.scalar.activation(out=p_nat[:, 512 * j:512 * (j + 1)],
                             in_=ps_sc[32 * j:32 * j + H, 0:512], func=AF.Exp,
                             scale=1.0 / SU)
    den = singles.tile([H, 1], F32)
    nc.vector.reduce_sum(out=den[:], in_=p_nat[:], axis=mybir.AxisListType.X)
    nc.vector.reciprocal(out=den[:], in_=den[:])
    pT = singles.tile([P, SC, H], F16)
    t_nat_to_T(p_nat, pT, SC, H, "p")

    # =======================================================================
    # STEP 3: pooled (8h, 4096d) = pT^T @ llm/den  [f16, tiles-over-n]
    # =======================================================================
    ps_pool = psA.tile([P, 1024], F32, tag="psA", name="ps_pool")
    for c in range(SC):
        lt = ln16p.tile([P, D], F16, tag="ln16", name=f"ln16_{c}")
        nc.scalar.dma_start(out=lt[:], in_=t["llm16"][c * P:(c + 1) * P, :])
        for j in range(4):
            for u in range(2):
                nc.tensor.matmul(
                    ps_pool[32 * j:32 * j + H, 512 * u:512 * (u + 1)],
                    pT[:, c, :],
                    lt[:, 1024 * j + 512 * u:1024 * j + 512 * (u + 1)],
                    start=(c == 0), stop=(c == SC - 1),
                    tile_position=(0, 32 * j))
    pooled_nat = natp.tile([H, D], F16, tag="natD", name="pooled_nat")
    for j in range(4):
        nc.vector.tensor_scalar(
            out=pooled_nat[:, 1024 * j:1024 * (j + 1)],
            in0=ps_pool[32 * j:32 * j + H, :],
            scalar1=den[:], scalar2=None, op0=ALU.mult)
    nc.gpsimd.dma_start(out=t["cc_pool_in"][:], in_=pooled_nat[:])
    nc.gpsimd.collective_compute(
        "AllToAll", ALU.bypass, replica_groups=GROUPS,
        ins=[t["cc_pool_in"][:].opt()], outs=[t["cc_pool_out"][:].opt()])
    poolh_nat = natp.tile([B, D], F16, tag="natD", name="poolh_nat")
    nc.gpsimd.dma_start(out=poolh_nat[:], in_=t["cc_pool_out"][:])

    # =======================================================================
    # STEP 4: ctx (8b, 512) = poolh @ wv + bv  [tiles-over-k]
    # =======================================================================
    poolhT = singles.tile([P, DC, B], F16)
    t_nat_to_T(poolh_nat, poolhT, DC, B, "ph")
    ps_cx = psB.tile([P, 512], F32, tag="psB", name="ps_cx")
    for g in range(8):
        for cc in range(4):
            c = 4 * g + cc
            j = c % 4
            nc.tensor.matmul(ps_cx[32 * j:32 * j + B, :],
                             poolhT[:, c, :], wv_tiles[g][:, cc, :],
                             start=(c < 4), stop=(c >= DC - 4 and j != 0),
                             tile_position=(0, 32 * j))
    bias_rep(ps_cx[0:B, :], BV_O, DH, (0, 0), stop=True)
    ctx_nat = natp.tile([B, DH], F16, tag="nat8", name="ctx_nat")
    quad_sum(ctx_nat[:], ps_cx, DH)
    ctxT = singles.tile([P, DH // P, B], F16)
    t_nat_to_T(ctx_nat, ctxT, DH // P, B, "cx")

    # =======================================================================
    # STEP 5: attn partial = ctx @ wo + bo/8 ; AllReduce(f32 wire)
    # =======================================================================
    wo_tiles = []
    for c in range(4):
        wt = wop.tile([P, D], F16, tag="wo", name=f"wo_{c}")
        nc.gpsimd.dma_start(out=wt[:], in_=t["wo16"][c * P:(c + 1) * P, :])
        wo_tiles.append(wt)
    ps_at = psA.tile([P, 1024], F32, tag="psA", name="ps_at")
    for c in range(4):
        for j in range(4):
            for u in range(2):
                n0 = 1024 * j + 512 * u
                nc.tensor.matmul(
                    ps_at[32 * j:32 * j + B, 512 * u:512 * (u + 1)],
                    ctxT[:, c, :], wo_tiles[c][:, n0:n0 + 512],
                    start=(c == 0), stop=False,
                    tile_position=(0, 32 * j))
    for j in range(4):
        bias_rep(ps_at[32 * j:32 * j + B, :], BO_O + 1024 * j, 1024,
                 (0, 32 * j), stop=True)
    attn_part = natp.tile([B, D], F16, tag="natD", name="attn_part")
    for j in range(4):
        nc.scalar.activation(out=attn_part[:, 1024 * j:1024 * (j + 1)],
                             in_=ps_at[32 * j:32 * j + B, :],
                             func=AF.Identity)
    nc.gpsimd.dma_start(out=t["cc_attn_in"][:], in_=attn_part[:])  # casts f32
    nc.gpsimd.collective_compute(
        "AllReduce", ALU.add, replica_groups=GROUPS,
        ins=[t["cc_attn_in"][:].opt()], outs=[t["cc_attn_out"][:].opt()])

    # overlapped with the AllReduce: x0 += attn_partial @ rin_pool
    apT = singles.tile([P, DC, B], F16)
    t_nat_to_T(attn_part, apT, DC, B, "ap", evict_eng=nc.scalar)
    ps_xa = psB.tile([P, 512], F32, tag="psB", name="ps_xa")
    for c in range(DC):
        j = c % 4
        nc.tensor.matmul(ps_xa[32 * j:32 * j + B, :HID], apT[:, c, :],
                         rp_sb[:, c, :], start=(c < 4), stop=(c >= DC - 4),
                         tile_position=(0, 32 * j))
    xa_nat = singles.tile([B, HID], F32)
    quad_sum(xa_nat[:], ps_xa, HID)
    nc.vector.tensor_add(out=x0_early[:], in0=x0_early[:], in1=xa_nat[:])

    # =======================================================================
    # STEP 6: y = LN(attn)*g+b ; mm1: g1 = gelu(y@w1 + b1)
    # =======================================================================
    attn_nat = natp.tile([B, D], F16, tag="natD", name="attn_nat")
    nc.gpsimd.dma_start(out=attn_nat[:], in_=t["cc_attn_out"][:])  # f32->f16
    y_nat = natp.tile([B, D], F16, tag="natD", name="y_nat")
    layernorm_nat(attn_nat[:], B, D, y_nat[:], "ln0")
    yT = singles.tile([P, DC, B], F16)
    t_nat_to_T(y_nat, yT, DC, B, "y")
    for c in range(DC):
        nc.vector.tensor_scalar(out=yT[:, c, :], in0=yT[:, c, :],
                                scalar1=lng_sb[:, c:c + 1],
                                scalar2=lnb_sb[:, c:c + 1],
                                op0=ALU.mult, op1=ALU.add)

    ps_h1 = psA.tile([P, 1024], F32, tag="psA", name="ps_h1")
    for g in range(16):
        wt = w1p.tile([P, 2, F1S], F16, tag="w1", name=f"w1_{g}")
        nc.sync.dma_start(out=wt[:], in_=t["w116"][g])
        for cc in range(2):
            c = 2 * g + cc
            for j in range(4):
                nc.tensor.matmul(
                    ps_h1[32 * j:32 * j + B, 0:512],
                    yT[:, c, :], wt[:, cc, 512 * j:512 * (j + 1)],
                    start=(c == 0), stop=False,
                    tile_position=(0, 32 * j))
    for j in range(4):
        bias_rep(ps_h1[32 * j:32 * j + B, 0:512], B1_O + 512 * j, 512,
                 (0, 32 * j), stop=True)
    g_nat = natp.tile([B, F1S], F16, tag="nat8", name="g_nat")
    for j in range(4):
        nc.scalar.activation(out=g_nat[:, 512 * j:512 * (j + 1)],
                             in_=ps_h1[32 * j:32 * j + B, 0:512],
                             func=AF.Gelu)
    gT = singles.tile([P, FC, B], F16)
    t_nat_to_T(g_nat, gT, FC, B, "g")

    # =======================================================================
    # STEP 7: x0 += g1 @ w2rin ; AllReduce(x0)  [8 KiB]
    # =======================================================================
    w2r_sb = w1p.tile([P, FC, HID], F16, tag="w2r", bufs=1, name="w2r")
    nc.sync.dma_start(out=w2r_sb[:], in_=t["w2rin16"][:])
    ps_x0 = psB.tile([P, 512], F32, tag="psB", name="ps_x0")
    for c in range(FC):
        j = c % 4
        nc.tensor.matmul(ps_x0[32 * j:32 * j + B, :HID], gT[:, c, :],
                         w2r_sb[:, c, :], start=(c < 4), stop=(c >= FC - 4),
                         tile_position=(0, 32 * j))
    xg_nat = singles.tile([B, HID], F32)
    quad_sum(xg_nat[:], ps_x0, HID)
    nc.vector.tensor_add(out=x0_early[:], in0=x0_early[:], in1=xg_nat[:])
    nc.gpsimd.dma_start(out=t["cc_x0_in"][:], in_=x0_early[:])
    nc.gpsimd.collective_compute(
        "AllReduce", ALU.add, replica_groups=GROUPS,
        ins=[t["cc_x0_in"][:].opt()], outs=[t["cc_x0_out"][:].opt()])

    # =======================================================================
    # STEP 8: diffusion tail (replicated)
    # =======================================================================
    bw1a = w1p.tile([P, 3, 4 * HID], F16, tag="w1", name="bw1a")
    nc.sync.dma_start(out=bw1a[:], in_=t["bw1a"][:])
    bw1b = w1p.tile([P, 3, 4 * HID], F16, tag="w1", name="bw1b")
    nc.sync.dma_start(out=bw1b[:], in_=t["bw1b"][:])
    bw2a = w1p.tile([P, 12, HID], F16, tag="w1", name="bw2a")
    nc.sync.dma_start(out=bw2a[:], in_=t["bw2a"][:])
    bw2b = w1p.tile([P, 12, HID], F16, tag="w1", name="bw2b")
    nc.sync.dma_start(out=bw2b[:], in_=t["bw2b"][:])

    x_nat = singles.tile([B, HID], F32)
    nc.gpsimd.dma_start(out=x_nat[:], in_=t["cc_x0_out"][:])

    for i in range(NBLK):
        xn = singles.tile([B, HID], F16, name=f"xn_{i}")
        layernorm_nat(x_nat[:], B, HID, xn[:], f"lnb{i}")
        xnT = singles.tile([P, HC, B], F16, name=f"xnT_{i}")
        t_nat_to_T(xn, xnT, HC, B, f"xn{i}")
        for c in range(HC):
            nc.vector.tensor_scalar(out=xnT[:, c, :], in0=xnT[:, c, :],
                                    scalar1=bgr_sb[:, i, c:c + 1],
                                    scalar2=bbr_sb[:, i, c:c + 1],
                                    op0=ALU.mult, op1=ALU.add)
        ps_bh = psB.tile([P, 512], F32, tag="psB", name=f"ps_bh_{i}")
        for j in range(4):
            for c in range(HC):
                f = 2 * i + c
                src = bw1a if f < 3 else bw1b
                nc.tensor.matmul(
                    ps_bh[32 * j:32 * j + B, 0:256],
                    xnT[:, c, :],
                    src[:, f % 3, 256 * j:256 * (j + 1)],
                    start=(c == 0), stop=False,
                    tile_position=(0, 32 * j))
            bias_rep(ps_bh[32 * j:32 * j + B, 0:256],
                     BB1_O + 1024 * i + 256 * j, 256, (0, 32 * j), stop=True)
        hb = natp.tile([B, 4 * HID], F16, tag="nat8", name=f"hb_{i}")
        for j in range(4):
            nc.scalar.activation(out=hb[:, 256 * j:256 * (j + 1)],
                                 in_=ps_bh[32 * j:32 * j + B, 0:256],
                                 func=AF.Silu)
        hbT = singles.tile([P, 4 * HID // P, B], F16, name=f"hbT_{i}")
        t_nat_to_T(hb, hbT, 4 * HID // P, B, f"hb{i}")

        ps_bo = psB.tile([P, 512], F32, tag="psB", name=f"ps_bo_{i}")
        for c in range(4 * HID // P):
            j = c % 4
            f = 8 * i + c
            src = bw2a if f < 12 else bw2b
            nc.tensor.matmul(ps_bo[32 * j:32 * j + B, :HID], hbT[:, c, :],
                             src[:, f % 12, :],
                             start=(c < 4), stop=(c >= 4 and j != 0),
                             tile_position=(0, 32 * j))
        bias_rep(ps_bo[0:B, :HID], BB2_O + 256 * i, HID, (0, 0), stop=True)
        xr = singles.tile([B, HID], F32, name=f"xr_{i}")
        quad_sum(xr[:], ps_bo, HID)
        nc.vector.tensor_add(out=x_nat[:], in0=x_nat[:], in1=xr[:])

    xs = singles.tile([B, HID], F16)
    nc.scalar.activation(out=xs[:], in_=x_nat[:], func=AF.Silu)
    xsT = singles.tile([P, HC, B], F16)
    t_nat_to_T(xs, xsT, HC, B, "xs")
    ps_o = psB.tile([P, 512], F32, tag="psB", name="ps_o")
    for c in range(HC):
        nc.tensor.matmul(ps_o[:B, :AD], xsT[:, c, :], ow_sb[:, c, :],
                         start=(c == 0), stop=(c == HC - 1))
    out_sb = singles.tile([B, AD], F32)
    nc.vector.tensor_add(out=out_sb[:], in0=ps_o[:B, :AD], in1=ob_bc[:])
    nc.sync.dma_start(out=t["res"][:], in_=out_sb[:])


_CACHED_NC = None


def _get_nc():
    global _CACHED_NC
    if _CACHED_NC is None:
        _CACHED_NC = build_program()
    return _CACHED_NC


def _prep_in_maps(inputs):
    f32 = np.float32
    f16 = np.float16
    llm_full = np.asarray(inputs["llm_output"], dtype=f32)
    wq = np.asarray(inputs["wq"], f32); wk = np.asarray(inputs["wk"], f32)
    wv = np.asarray(inputs["wv"], f32); wo = np.asarray(inputs["wo"], f32)
    bq = np.asarray(inputs["bq"], f32); bv = np.asarray(inputs["bv"], f32)
    bo = np.asarray(inputs["bo"], f32)
    w1 = np.asarray(inputs["mlp_w1"], f32); b1 = np.asarray(inputs["mlp_b1"], f32)
    w2 = np.asarray(inputs["mlp_w2"], f32); b2 = np.asarray(inputs["mlp_b2"], f32)
    rin_w = np.asarray(inputs["rin_w"], f32)
    rin_b = np.asarray(inputs["rin_b"], f32)
    probe = np.asarray(inputs["probe"], f32).reshape(D)
    cw2 = np.asarray(inputs["cond_w2"], f32)
    cb2 = np.asarray(inputs["cond_b2"], f32)
    blk_g = np.asarray(inputs["blk_ln_g"], f32)
    blk_b = np.asarray(inputs["blk_ln_b"], f32)
    blk_w1 = np.asarray(inputs["blk_w1"], f32)
    blk_w2 = np.asarray(inputs["blk_w2"], f32)
    blk_b1 = np.asarray(inputs["blk_b1"], f32)
    blk_b2 = np.asarray(inputs["blk_b2"], f32)

    # ---- weight-only folds ----
    q = (probe @ wq + bq) * RSQRT_DH
    U = np.zeros((D, H), f32)
    for h in range(H):
        U[:, h] = wk[:, h * DH:(h + 1) * DH] @ q[h * DH:(h + 1) * DH]
    U8 = (U * SU).astype(NP8)
    rin_cond = rin_w[0:TD]
    rin_pool = np.ascontiguousarray(rin_w[TD:TD + D])
    rin_na = rin_w[TD + D:]
    w2rin = w2 @ rin_pool                      # (4D, HID), ~17 GFLOP on host
    cw2rin = cw2 @ rin_cond
    rb_fold = (rin_b + b2 @ rin_pool + cb2 @ rin_cond) / NC

    def r128(v):
        return np.ascontiguousarray(v.reshape(-1, P).T)

    def ptile(m, c_per_g):
        K, N = m.shape
        G = K // (P * c_per_g)
        r = np.ascontiguousarray(
            m.reshape(G, c_per_g, P, N).transpose(0, 2, 1, 3))
        return r if G > 1 else r[0]

    shared = {
        "rp16": np.ascontiguousarray(
            rin_pool.reshape(DC, P, HID).transpose(1, 0, 2)).astype(f16),
        "ln_g_r": r128(np.asarray(inputs["ln_g"], f32)),
        "ln_b_r": r128(np.asarray(inputs["ln_b"], f32)),
        "four_w2": np.concatenate(
            [np.asarray(inputs["four_w"], f32).reshape(TD // 2, 1)] * 2),
        "phase2": np.concatenate(
            [np.full((TD // 2, 1), np.pi / 2, f32),
             np.zeros((TD // 2, 1), f32)]),
        "timeT": np.ascontiguousarray(np.asarray(inputs["time"], f32).T),
        "naT": np.ascontiguousarray(
            np.asarray(inputs["noisy_actions"], f32).T).astype(f16),
        "cw1": np.asarray(inputs["cond_w1"], f32).astype(f16),
        "cb1c": np.asarray(inputs["cond_b1"], f32).reshape(-1, 1),
        "cw2rin8": (cw2rin / NC).astype(f16),
        "rna8": (rin_na / NC).astype(f16),
        "rb8": rb_fold.astype(f16).reshape(1, HID),
        "bgr": np.ascontiguousarray(
            blk_g.reshape(NBLK, HC, P).transpose(2, 0, 1)),
        "bbr": np.ascontiguousarray(
            blk_b.reshape(NBLK, HC, P).transpose(2, 0, 1)),
        "bw1a": np.ascontiguousarray(
            blk_w1.reshape(NBLK * HC, P, 4 * HID)[0:3].transpose(1, 0, 2)
        ).astype(f16),
        "bw1b": np.ascontiguousarray(
            blk_w1.reshape(NBLK * HC, P, 4 * HID)[3:6].transpose(1, 0, 2)
        ).astype(f16),
        "bw2a": np.ascontiguousarray(
            blk_w2.reshape(NBLK * 8, P, HID)[0:12].transpose(1, 0, 2)
        ).astype(f16),
        "bw2b": np.ascontiguousarray(
            blk_w2.reshape(NBLK * 8, P, HID)[12:24].transpose(1, 0, 2)
        ).astype(f16),
        "ow": np.ascontiguousarray(
            np.asarray(inputs["out_w"], f32).reshape(HC, P, AD)
            .transpose(1, 0, 2)).astype(f16),
        "out_bc": np.asarray(inputs["out_b"], f32).reshape(1, AD),
        "U8r": np.ascontiguousarray(U8.reshape(DC, P, H).transpose(1, 0, 2)),
    }

    in_maps = []
    for i in range(NC):
        hb_ = slice(i * DH, (i + 1) * DH)
        fb = slice(i * F1S, (i + 1) * F1S)
        m = dict(shared)
        m["llm16"] = llm_full[i].astype(f16)
        m["llmT8"] = np.ascontiguousarray(llm_full[i].T).astype(NP8)
        m["wv16"] = ptile(np.ascontiguousarray(wv[:, hb_]), 4).astype(f16)
        m["wo16"] = np.ascontiguousarray(wo[hb_, :]).astype(f16)
        m["w116"] = ptile(np.ascontiguousarray(w1[:, fb]), 2).astype(f16)
        m["w2rin16"] = ptile(np.ascontiguousarray(w2rin[fb]), FC).astype(f16)
        brep = np.zeros((1, BREP_N), f16)
        brep[0, BO_O:BO_O + D] = (bo / NC).astype(f16)
        brep[0, B1_O:B1_O + F1S] = b1[fb].astype(f16)
        brep[0, BV_O:BV_O + DH] = bv[hb_].astype(f16)
        brep[0, BB1_O:BB1_O + NBLK * 4 * HID] = blk_b1.reshape(-1).astype(f16)
        brep[0, BB2_O:BB2_O + NBLK * HID] = blk_b2.reshape(-1).astype(f16)
        m["brep"] = brep
        in_maps.append(m)
    return in_maps


def kernel(**inputs):
    nc = _get_nc()
    in_maps = _prep_in_maps(inputs)
    r = run_bass_kernel_spmd(nc, in_maps, core_ids=list(range(NC)))
    return np.ascontiguousarray(r.results[0]["res"]).astype(np.float32)


def run_traced(**inputs):
    nc = _get_nc()
    in_maps = _prep_in_maps(inputs)
    r = run_bass_kernel_spmd(nc, in_maps, core_ids=list(range(NC)), trace=True)
    return np.ascontiguousarray(r.results[0]["res"]).astype(np.float32), r


# revision 14
# speedup vs baseline: 1.4548x; 1.0639x over previous
"""Trainium2 Bass kernel for nn_DiffusionActionHead (B=8, S=2048, D=4096).

v3 strategy (8 NeuronCores, batch-parallel + head-parallel):
  - Host folds weight-only math:  U = wk^T (probe@wq + bq) / sqrt(DH)
    (removes wq/wk and the U AllGather);  w2rin = mlp_w2 @ rin_w[pool]
    ((attn_out+h) is consumed only through rin_w -> the 16 MiB w2 stream
    becomes 1 MiB and the mlp AllReduce becomes the 8 KiB x0 AllReduce);
    LN affine gains fold into w1 / blk_w1 rows (y_aff@W = y_core@(g*W) +
    (b@W folded into the bias)).
  - Scores stream llm^T in fp8 e3m4 (softmax washes the quantization to
    ~0.2% on attention weights); pooled streams llm natural in f16.
  - All m=8 matmuls 4-way column-tiled (tile_position, measured 2.35x).
  - Pooled runs in two D-halves with two pipelined AllToAlls; ctx
    consumes each half as it lands.  x0 partials (attn_part@rin_pool,
    computed during the attn AllReduce) collapse into one 8 KiB AR.
  - Biases enter PSUM via 128-row replicated bias tile (ones/128) so all
    matmuls keep the (128,32) PE tiling mode.
  - Rings: scalar = llm streams + wo + odd w1; sync = rin_pool + even w1
    + w2rin + tail weights; gpsimd = smalls, wv, collective bounces
    (with f16<->f32 casts on the attn AllReduce wire).
"""

import numpy as np
import sys

if "/opt/trn_rl_repo" not in sys.path:
    sys.path.insert(0, "/opt/trn_rl_repo")

import ml_dtypes
import concourse.bass as bass
import concourse.tile as tile
from concourse import bacc, mybir
from concourse.masks import make_identity
from concourse.bass_utils import run_bass_kernel_spmd

F32 = mybir.dt.float32
F16 = mybir.dt.float16
F8 = mybir.dt.float8e3
NP8 = ml_dtypes.float8_e3m4
AF = mybir.ActivationFunctionType
ALU = mybir.AluOpType

B, S, D = 8, 2048, 4096
H, AD, TD, HID, NBLK = 8, 7, 32, 256, 3
DH = D // H
NC = 8
P = 128
SC = S // P            # 16
DC = D // P            # 32
HD2 = D // 2           # 2048
F1S = 4 * D // NC      # 2048
FC = F1S // P          # 16
HC = HID // P          # 2
SU = 2048.0
RSQRT_DH = 1.0 / float(np.sqrt(DH))
TWO_PI = 2.0 * float(np.pi)

BO_O = 0
B1_O = 4096
BV_O = 6144
BB1_O = 6656
BB2_O = 9728
BREP_N = 10496


def _bcast(src_ap, nparts):
    ap = src_ap
    assert ap.shape[0] == 1, ap.shape
    return bass.AP(tensor=ap.tensor, offset=ap.offset,
                   ap=[[0, nparts]] + [list(x) for x in ap.ap[1:]])


def build_program():
    nc = bacc.Bacc("TRN2", target_bir_lowering=False, debug=False,
                   num_devices=NC)
    t = {}

    def din(name, shape, dtype=F32):
        t[name] = nc.dram_tensor(name, shape, dtype, kind="ExternalInput")

    din("llm16", [S, D], F16)
    din("llmT8", [D, S], F8)
    din("U8r", [P, DC, H], F8)
    din("wv16", [8, P, 4, DH], F16)
    din("wo16", [DH, D], F16)
    din("w116", [16, P, 2, F1S], F16)
    din("w2rin16", [P, FC, HID], F16)
    din("rp16", [P, DC, HID], F16)
    din("brep", [1, BREP_N], F16)
    din("four_w2", [TD, 1]); din("phase2", [TD, 1])
    din("timeT", [1, B]); din("naT", [AD, B], F16)
    din("cw1", [TD, 2 * TD], F16); din("cb1c", [2 * TD, 1])
    din("cw2rin8", [2 * TD, HID], F16)
    din("rna8", [AD, HID], F16)
    din("rb8", [1, HID], F16)
    din("bw1a", [P, 3, 4 * HID], F16)
    din("bw1b", [P, 3, 4 * HID], F16)
    din("bw2a", [P, 12, HID], F16)
    din("bw2b", [P, 12, HID], F16)
    din("ow", [P, HC, AD], F16); din("out_bc", [1, AD])
    t["res"] = nc.dram_tensor("res", [B, AD], F32, kind="ExternalOutput")

    for hf in range(2):
        t[f"cc_pool_in{hf}"] = nc.dram_tensor(f"cc_pool_in{hf}", [H, HD2], F16)
        t[f"cc_pool_out{hf}"] = nc.dram_tensor(f"cc_pool_out{hf}", [B, HD2],
                                               F16)
    t["cc_attn_in"] = nc.dram_tensor("cc_attn_in", [B, D], F32)
    t["cc_attn_out"] = nc.dram_tensor("cc_attn_out", [B, D], F32,
                                      addr_space="Shared")
    t["cc_x0_in"] = nc.dram_tensor("cc_x0_in", [B, HID], F32)
    t["cc_x0_out"] = nc.dram_tensor("cc_x0_out", [B, HID], F32,
                                    addr_space="Shared")

    with tile.TileContext(nc) as tc:
        import contextlib
        with contextlib.ExitStack() as ctx:
            _build(nc, tc, t, ctx)
    nc.finalize()
    return nc


def _build(nc, tc, t, ctx):
    GROUPS = [list(range(NC))]

    singles = ctx.enter_context(tc.tile_pool(name="singles", bufs=1))
    lt8p = ctx.enter_context(tc.tile_pool(name="lt8p", bufs=2))
    ln16p = ctx.enter_context(tc.tile_pool(name="ln16p", bufs=4))
    natp = ctx.enter_context(tc.tile_pool(name="natp", bufs=2))
    wvp = ctx.enter_context(tc.tile_pool(name="wvp", bufs=8))
    wop = ctx.enter_context(tc.tile_pool(name="wop", bufs=3))
    w1p = ctx.enter_context(tc.tile_pool(name="w1p", bufs=4))
    psA = ctx.enter_context(tc.tile_pool(name="psA", bufs=2, space="PSUM"))
    psB = ctx.enter_context(tc.tile_pool(name="psB", bufs=2, space="PSUM"))
    psT8 = ctx.enter_context(tc.tile_pool(name="psT8", bufs=2, space="PSUM"))

    ident = singles.tile([P, P], F32)
    make_identity(nc, ident)
    ident16 = singles.tile([P, P], F16)
    nc.vector.tensor_copy(out=ident16[:], in_=ident[:])
    eps_sb = singles.tile([P, 1], F32)
    nc.vector.memset(eps_sb[:], 1e-5)
    ones8 = singles.tile([1, 8], F16)
    nc.vector.memset(ones8[:], 1.0)
    ones128 = singles.tile([P, 8], F16)
    nc.vector.memset(ones128[:], 1.0 / 128.0)

    def t_nat_to_T(src_nat, dst_T, nchunks, npart, uid, evict_eng=None,
                   c0=0):
        eng = evict_eng or nc.vector
        for c in range(nchunks):
            ps = psT8.tile([P, 16], F16, tag="tp16", name=f"tp_{uid}_{c}")
            nc.tensor.transpose(ps[:, :npart], src_nat[:, c * P:(c + 1) * P],
                                ident16[:npart, :npart])
            if eng is nc.scalar:
                nc.scalar.activation(out=dst_T[:, c0 + c, :],
                                     in_=ps[:, :npart], func=AF.Identity)
            else:
                eng.tensor_copy(out=dst_T[:, c0 + c, :], in_=ps[:, :npart])

    def bias_rep(ps_slice, col0, n_total, tp, stop):
        nchn = (n_total + 511) // 512
        for n in range(nchn):
            w = min(512, n_total - n * 512)
            nc.tensor.matmul(
                ps_slice[:, n * 512:n * 512 + w], ones128[:, :B],
                brep_sb[:, col0 + n * 512:col0 + n * 512 + w],
                start=False, stop=stop, tile_position=tp)

    def quad_sum(dst, ps, n):
        nc.vector.tensor_copy(out=dst, in_=ps[0:B, :n])
        nc.vector.tensor_add(out=dst, in0=dst, in1=ps[32:32 + B, :n])
        nc.vector.tensor_add(out=dst, in0=dst, in1=ps[64:64 + B, :n])
        nc.vector.tensor_add(out=dst, in0=dst, in1=ps[96:96 + B, :n])

    def layernorm_nat(x_nat, npart, n, y_nat, uid):
        nsub = max(1, n // 512)
        st = singles.tile([npart, nsub, nc.vector.BN_STATS_DIM], F32,
                          name=f"lnst_{uid}")
        xg = x_nat.rearrange("p (a b) -> p a b", a=nsub)
        for g in range(nsub):
            nc.vector.bn_stats(out=st[:, g, :], in_=xg[:, g, :])
        mv = singles.tile([npart, nc.vector.BN_AGGR_DIM], F32,
                          name=f"lnmv_{uid}")
        nc.vector.bn_aggr(out=mv[:], in_=st[:])
        std = singles.tile([npart, 1], F32, name=f"lnsd_{uid}")
        nc.scalar.activation(out=std[:], in_=mv[:, 1:2], func=AF.Sqrt,
                             bias=eps_sb[:npart, :])
        nc.vector.reciprocal(out=std[:], in_=std[:])
        nc.vector.tensor_scalar(out=y_nat, in0=x_nat, scalar1=mv[:, 0:1],
                                scalar2=std[:], op0=ALU.subtract, op1=ALU.mult)

    # ===== STEP 0: U + rp on sync; llmT8 stream hoisted on scalar; smalls
    # on gpsimd (cond inputs first, bulky brep last).
    u8_sb = singles.tile([P, DC, H], F8)
    nc.sync.dma_start(out=u8_sb[:], in_=t["U8r"][:])
    rp_sb = singles.tile([P, DC, HID], F16)
    nc.sync.dma_start(out=rp_sb[:], in_=t["rp16"][:])

    lt_tiles = []
    for g in range(8):
        lt = lt8p.tile([P, 4, S], F8, tag="lt8", name=f"lt8_{g}")
        nc.scalar.dma_start(
            out=lt[:],
            in_=t["llmT8"][g * 512:(g + 1) * 512, :].rearrange(
                "(c p) s -> p c s", p=P))
        lt_tiles.append(lt)

    fw_sb = singles.tile([TD, 1], F32)
    nc.gpsimd.dma_start(out=fw_sb[:], in_=t["four_w2"][:])
    ph_sb = singles.tile([TD, 1], F32)
    nc.gpsimd.dma_start(out=ph_sb[:], in_=t["phase2"][:])
    tb32 = singles.tile([TD, B], F32)
    nc.gpsimd.dma_start(out=tb32[:], in_=_bcast(t["timeT"][:], TD))
    cw1_sb = singles.tile([TD, 2 * TD], F16)
    nc.gpsimd.dma_start(out=cw1_sb[:], in_=t["cw1"][:])
    cb1_sb = singles.tile([2 * TD, 1], F32)
    nc.gpsimd.dma_start(out=cb1_sb[:], in_=t["cb1c"][:])
    cwr_sb = singles.tile([2 * TD, HID], F16)
    nc.gpsimd.dma_start(out=cwr_sb[:], in_=t["cw2rin8"][:])
    naT_sb = singles.tile([AD, B], F16)
    nc.gpsimd.dma_start(out=naT_sb[:], in_=t["naT"][:])
    rna_sb = singles.tile([AD, HID], F16)
    nc.gpsimd.dma_start(out=rna_sb[:], in_=t["rna8"][:])
    rb_sb = singles.tile([1, HID], F16)
    nc.gpsimd.dma_start(out=rb_sb[:], in_=t["rb8"][:])
    wv_tiles = []
    for g in range(8):
        wt = wvp.tile([P, 4, DH], F16, tag="wv", name=f"wv_{g}")
        nc.gpsimd.dma_start(out=wt[:], in_=t["wv16"][g])
        wv_tiles.append(wt)
    brep_sb = singles.tile([P, BREP_N], F16)
    nc.gpsimd.dma_start(out=brep_sb[:], in_=_bcast(t["brep"][:], P))
    ow_sb = singles.tile([P, HC, AD], F16)
    nc.gpsimd.dma_start(out=ow_sb[:], in_=t["ow"][:])
    ob_bc = singles.tile([B, AD], F32)
    nc.gpsimd.dma_start(out=ob_bc[:], in_=_bcast(t["out_bc"][:], B))

    # ===== STEP 1: scoresT = (U*SU)^T @ llmT  [fp8, tiles-over-n]
    ps_sc = psA.tile([P, 1024], F32, tag="psA", name="ps_sc")
    for g in range(8):
        for cc in range(4):
            c = 4 * g + cc
            for j in range(4):
                nc.tensor.matmul(
                    ps_sc[32 * j:32 * j + H, 0:512],
                    u8_sb[:, c, :], lt_tiles[g][:, cc, 512 * j:512 * (j + 1)],
                    start=(c == 0), stop=(c == DC - 1),
                    tile_position=(0, 32 * j))

    p_nat = natp.tile([H, S], F16, tag="nat8", name="p_nat")
    for j in range(4):
        nc.scalar.activation(out=p_nat[:, 512 * j:512 * (j + 1)],
                             in_=ps_sc[32 * j:32 * j + H, 0:512], func=AF.Exp,
                             scale=1.0 / SU)
    den = singles.tile([H, 1], F32)
    nc.vector.reduce_sum(out=den[:], in_=p_nat[:], axis=mybir.AxisListType.X)
    nc.vector.reciprocal(out=den[:], in_=den[:])
    pT = singles.tile([P, SC, H], F16)
    t_nat_to_T(p_nat, pT, SC, H, "p")

    # ===== STEP 2: pooled in two D-halves, AllToAll pipelined per half
    poolh = []
    for hf in range(2):
        ps_pool = psA.tile([P, 1024], F32, tag="psA", name=f"ps_pool{hf}")
        for c in range(SC):
            lt = ln16p.tile([P, HD2], F16, tag="ln16", name=f"ln16_{hf}_{c}")
            nc.scalar.dma_start(
                out=lt[:],
                in_=t["llm16"][c * P:(c + 1) * P, hf * HD2:(hf + 1) * HD2])
            for j in range(4):
                nc.tensor.matmul(
                    ps_pool[32 * j:32 * j + H, 0:512],
                    pT[:, c, :], lt[:, 512 * j:512 * (j + 1)],
                    start=(c == 0), stop=(c == SC - 1),
                    tile_position=(0, 32 * j))
        pooled = natp.tile([H, HD2], F16, tag="nat8", name=f"pooled{hf}")
        for j in range(4):
            nc.vector.tensor_scalar(
                out=pooled[:, 512 * j:512 * (j + 1)],
                in0=ps_pool[32 * j:32 * j + H, 0:512],
                scalar1=den[:], scalar2=None, op0=ALU.mult)
        nc.gpsimd.dma_start(out=t[f"cc_pool_in{hf}"][:], in_=pooled[:])
        nc.gpsimd.collective_compute(
            "AllToAll", ALU.bypass, replica_groups=GROUPS,
            ins=[t[f"cc_pool_in{hf}"][:].opt()],
            outs=[t[f"cc_pool_out{hf}"][:].opt()])
        ph_t = natp.tile([B, HD2], F16, tag="nat8", name=f"poolh{hf}")
        nc.gpsimd.dma_start(out=ph_t[:], in_=t[f"cc_pool_out{hf}"][:])
        poolh.append(ph_t)

    # ---- cond path (off critical path; PE slots in while streams run)
    fu = singles.tile([TD, B], F32)
    nc.vector.tensor_scalar_mul(out=fu[:], in0=tb32[:], scalar1=fw_sb[:])
    fi = singles.tile([TD, B], mybir.dt.int32)
    nc.vector.tensor_copy(out=fi[:], in_=fu[:])
    fif = singles.tile([TD, B], F32)
    nc.vector.tensor_copy(out=fif[:], in_=fi[:])
    nc.vector.tensor_sub(out=fu[:], in0=fu[:], in1=fif[:])
    ffT = singles.tile([TD, B], F16)
    nc.scalar.activation(out=ffT[:], in_=fu[:], func=AF.Sin,
                         scale=TWO_PI, bias=ph_sb[:])
    ps_c1 = psB.tile([P, 512], F32, tag="psB", name="ps_c1")
    nc.tensor.matmul(ps_c1[:2 * TD, :B], cw1_sb[:], ffT[:], start=True,
                     stop=True)
    c1 = singles.tile([2 * TD, B], F16)
    nc.scalar.activation(out=c1[:], in_=ps_c1[:2 * TD, :B], func=AF.Silu,
                         bias=cb1_sb[:])
    ps_e = psB.tile([P, 512], F32, tag="psB", name="ps_e")
    nc.tensor.matmul(ps_e[:B, :HID], c1[:], cwr_sb[:], start=True, stop=False)
    nc.tensor.matmul(ps_e[:B, :HID], naT_sb[:], rna_sb[:], start=False,
                     stop=False)
    nc.tensor.matmul(ps_e[:B, :HID], ones8[:, :B], rb_sb[:], start=False,
                     stop=True)
    x0_early = singles.tile([B, HID], F32)
    nc.vector.tensor_copy(out=x0_early[:], in_=ps_e[:B, :HID])

    # ===== STEP 3: ctx = poolh @ wv + bv  [tiles-over-k, half-pipelined]
    poolhT = singles.tile([P, DC, B], F16)
    ps_cx = psB.tile([P, 512], F32, tag="psB", name="ps_cx")
    for hf in range(2):
        t_nat_to_T(poolh[hf], poolhT, SC, B, f"ph{hf}", c0=hf * SC)
        for g in range(4 * hf, 4 * hf + 4):
            for cc in range(4):
                c = 4 * g + cc
                j = c % 4
                nc.tensor.matmul(ps_cx[32 * j:32 * j + B, :],
                                 poolhT[:, c, :], wv_tiles[g][:, cc, :],
                                 start=(c < 4),
                                 stop=(c >= DC - 4 and j != 0),
                                 tile_position=(0, 32 * j))
    bias_rep(ps_cx[0:B, :], BV_O, DH, (0, 0), stop=True)
    ctx_nat = natp.tile([B, DH], F16, tag="nat8", name="ctx_nat")
    quad_sum(ctx_nat[:], ps_cx, DH)
    ctxT = singles.tile([P, DH // P, B], F16)
    t_nat_to_T(ctx_nat, ctxT, DH // P, B, "cx")

    # ===== STEP 4: attn partial = ctx @ wo + bo/8 ; AllReduce (f32 wire)
    wo_tiles = []
    for c in range(4):
        wt = wop.tile([P, D], F16, tag="wo", name=f"wo_{c}")
        nc.gpsimd.dma_start(out=wt[:], in_=t["wo16"][c * P:(c + 1) * P, :])
        wo_tiles.append(wt)
    ps_at = psA.tile([P, 1024], F32, tag="psA", name="ps_at")
    for c in range(4):
        for j in range(4):
            for u in range(2):
                n0 = 1024 * j + 512 * u
                nc.tensor.matmul(
                    ps_at[32 * j:32 * j + B, 512 * u:512 * (u + 1)],
                    ctxT[:, c, :], wo_tiles[c][:, n0:n0 + 512],
                    start=(c == 0), stop=False,
                    tile_position=(0, 32 * j))
    for j in range(4):
        bias_rep(ps_at[32 * j:32 * j + B, :], BO_O + 1024 * j, 1024,
                 (0, 32 * j), stop=True)
    attn_part = natp.tile([B, D], F16, tag="natD", name="attn_part")
    for j in range(4):
        nc.scalar.activation(out=attn_part[:, 1024 * j:1024 * (j + 1)],
                             in_=ps_at[32 * j:32 * j + B, :],
                             func=AF.Identity)
    # w1 stream issues now: evens on sync (start immediately), odds on
    # scalar (right behind wo) — resident before mm1 needs them.
    w1_tiles = []
    for g in range(16):
        wt = w1p.tile([P, 2, F1S], F16, tag="w1", name=f"w1_{g}")
        nc.sync.dma_start(out=wt[:], in_=t["w116"][g])
        w1_tiles.append(wt)
    nc.gpsimd.dma_start(out=t["cc_attn_in"][:], in_=attn_part[:])
    nc.gpsimd.collective_compute(
        "AllReduce", ALU.add, replica_groups=GROUPS,
        ins=[t["cc_attn_in"][:].opt()], outs=[t["cc_attn_out"][:].opt()])

    # overlapped with the AllReduce: x0 += attn_partial @ rin_pool
    apT = singles.tile([P, DC, B], F16)
    t_nat_to_T(attn_part, apT, DC, B, "ap")
    ps_xa = psB.tile([P, 512], F32, tag="psB", name="ps_xa")
    for c in range(DC):
        j = c % 4
        nc.tensor.matmul(ps_xa[32 * j:32 * j + B, :HID], apT[:, c, :],
                         rp_sb[:, c, :], start=(c < 4), stop=(c >= DC - 4),
                         tile_position=(0, 32 * j))
    xa_nat = singles.tile([B, HID], F32)
    quad_sum(xa_nat[:], ps_xa, HID)
    nc.vector.tensor_add(out=x0_early[:], in0=x0_early[:], in1=xa_nat[:])

    # ===== STEP 5: y = LN(attn) (affine folded into w1) ; mm1
    attn_nat = natp.tile([B, D], F16, tag="natD", name="attn_nat")
    nc.gpsimd.dma_start(out=attn_nat[:], in_=t["cc_attn_out"][:])
    y_nat = natp.tile([B, D], F16, tag="natD", name="y_nat")
    layernorm_nat(attn_nat[:], B, D, y_nat[:], "ln0")
    yT = singles.tile([P, DC, B], F16)
    t_nat_to_T(y_nat, yT, DC, B, "y")

    ps_h1 = psA.tile([P, 1024], F32, tag="psA", name="ps_h1")
    for g in range(16):
        for cc in range(2):
            c = 2 * g + cc
            for j in range(4):
                nc.tensor.matmul(
                    ps_h1[32 * j:32 * j + B, 0:512],
                    yT[:, c, :], w1_tiles[g][:, cc, 512 * j:512 * (j + 1)],
                    start=(c == 0), stop=False,
                    tile_position=(0, 32 * j))
    for j in range(4):
        bias_rep(ps_h1[32 * j:32 * j + B, 0:512], B1_O + 512 * j, 512,
                 (0, 32 * j), stop=True)
    g_nat = natp.tile([B, F1S], F16, tag="nat8", name="g_nat")
    for j in range(4):
        nc.scalar.activation(out=g_nat[:, 512 * j:512 * (j + 1)],
                             in_=ps_h1[32 * j:32 * j + B, 0:512],
                             func=AF.Gelu)
    gT = singles.tile([P, FC, B], F16)
    t_nat_to_T(g_nat, gT, FC, B, "g")

    # ===== STEP 6: x0 += g1 @ w2rin ; AllReduce(x0)
    w2r_sb = w1p.tile([P, FC, HID], F16, tag="w2r", bufs=1, name="w2r")
    nc.sync.dma_start(out=w2r_sb[:], in_=t["w2rin16"][:])
    ps_x0 = psB.tile([P, 512], F32, tag="psB", name="ps_x0")
    for c in range(FC):
        j = c % 4
        nc.tensor.matmul(ps_x0[32 * j:32 * j + B, :HID], gT[:, c, :],
                         w2r_sb[:, c, :], start=(c < 4), stop=(c >= FC - 4),
                         tile_position=(0, 32 * j))
    xg_nat = singles.tile([B, HID], F32)
    quad_sum(xg_nat[:], ps_x0, HID)
    nc.vector.tensor_add(out=x0_early[:], in0=x0_early[:], in1=xg_nat[:])
    nc.gpsimd.dma_start(out=t["cc_x0_in"][:], in_=x0_early[:])
    nc.gpsimd.collective_compute(
        "AllReduce", ALU.add, replica_groups=GROUPS,
        ins=[t["cc_x0_in"][:].opt()], outs=[t["cc_x0_out"][:].opt()])

    # ===== STEP 7: diffusion tail (replicated; blk LN affine folded)
    bw1a = w1p.tile([P, 3, 4 * HID], F16, tag="w1", name="bw1a")
    nc.sync.dma_start(out=bw1a[:], in_=t["bw1a"][:])
    bw1b = w1p.tile([P, 3, 4 * HID], F16, tag="w1", name="bw1b")
    nc.sync.dma_start(out=bw1b[:], in_=t["bw1b"][:])
    bw2a = w1p.tile([P, 12, HID], F16, tag="w1", name="bw2a")
    nc.sync.dma_start(out=bw2a[:], in_=t["bw2a"][:])
    bw2b = w1p.tile([P, 12, HID], F16, tag="w1", name="bw2b")
    nc.sync.dma_start(out=bw2b[:], in_=t["bw2b"][:])

    x_nat = singles.tile([B, HID], F32)
    nc.gpsimd.dma_start(out=x_nat[:], in_=t["cc_x0_out"][:])

    for i in range(NBLK):
        xn = singles.tile([B, HID], F16, name=f"xn_{i}")
        layernorm_nat(x_nat[:], B, HID, xn[:], f"lnb{i}")
        xnT = singles.tile([P, HC, B], F16, name=f"xnT_{i}")
        t_nat_to_T(xn, xnT, HC, B, f"xn{i}")
        ps_bh = psB.tile([P, 512], F32, tag="psB", name=f"ps_bh_{i}")
        for j in range(4):
            for c in range(HC):
                f = 2 * i + c
                src = bw1a if f < 3 else bw1b
                nc.tensor.matmul(
                    ps_bh[32 * j:32 * j + B, 0:256],
                    xnT[:, c, :], src[:, f % 3, 256 * j:256 * (j + 1)],
                    start=(c == 0), stop=False,
                    tile_position=(0, 32 * j))
            bias_rep(ps_bh[32 * j:32 * j + B, 0:256],
                     BB1_O + 1024 * i + 256 * j, 256, (0, 32 * j), stop=True)
        hb = natp.tile([B, 4 * HID], F16, tag="nat8", name=f"hb_{i}")
        for j in range(4):
            nc.scalar.activation(out=hb[:, 256 * j:256 * (j + 1)],
                                 in_=ps_bh[32 * j:32 * j + B, 0:256],
                                 func=AF.Silu)
        hbT = singles.tile([P, 4 * HID // P, B], F16, name=f"hbT_{i}")
        t_nat_to_T(hb, hbT, 4 * HID // P, B, f"hb{i}")

        ps_bo = psB.tile([P, 512], F32, tag="psB", name=f"ps_bo_{i}")
        for c in range(4 * HID // P):
            j = c % 4
            f = 8 * i + c
            src = bw2a if f < 12 else bw2b
            nc.tensor.matmul(ps_bo[32 * j:32 * j + B, :HID], hbT[:, c, :],
                             src[:, f % 12, :],
                             start=(c < 4), stop=(c >= 4 and j != 0),
                             tile_position=(0, 32 * j))
        bias_rep(ps_bo[0:B, :HID], BB2_O + 256 * i, HID, (0, 0), stop=True)
        xr = singles.tile([B, HID], F32, name=f"xr_{i}")
        quad_sum(xr[:], ps_bo, HID)
        nc.vector.tensor_add(out=x_nat[:], in0=x_nat[:], in1=xr[:])

    xs = singles.tile([B, HID], F16)
    nc.scalar.activation(out=xs[:], in_=x_nat[:], func=AF.Silu)
    xsT = singles.tile([P, HC, B], F16)
    t_nat_to_T(xs, xsT, HC, B, "xs")
    ps_o = psB.tile([P, 512], F32, tag="psB", name="ps_o")
    for c in range(HC):
        nc.tensor.matmul(ps_o[:B, :AD], xsT[:, c, :], ow_sb[:, c, :],
                         start=(c == 0), stop=(c == HC - 1))
    out_sb = singles.tile([B, AD], F32)
    nc.vector.tensor_add(out=out_sb[:], in0=ps_o[:B, :AD], in1=ob_bc[:])
    nc.sync.dma_start(out=t["res"][:], in_=out_sb[:])


_CACHED_NC = None


def _get_nc():
    global _CACHED_NC
    if _CACHED_NC is None:
        _CACHED_NC = build_program()
    return _CACHED_NC


def _prep_in_maps(inputs):
    f32 = np.float32
    f16 = np.float16
    llm_full = np.asarray(inputs["llm_output"], dtype=f32)
    wq = np.asarray(inputs["wq"], f32); wk = np.asarray(inputs["wk"], f32)
    wv = np.asarray(inputs["wv"], f32); wo = np.asarray(inputs["wo"], f32)
    bq = np.asarray(inputs["bq"], f32); bv = np.asarray(inputs["bv"], f32)
    bo = np.asarray(inputs["bo"], f32)
    ln_g = np.asarray(inputs["ln_g"], f32)
    ln_b = np.asarray(inputs["ln_b"], f32)
    w1 = np.asarray(inputs["mlp_w1"], f32); b1 = np.asarray(inputs["mlp_b1"], f32)
    w2 = np.asarray(inputs["mlp_w2"], f32); b2 = np.asarray(inputs["mlp_b2"], f32)
    rin_w = np.asarray(inputs["rin_w"], f32)
    rin_b = np.asarray(inputs["rin_b"], f32)
    probe = np.asarray(inputs["probe"], f32).reshape(D)
    cw2 = np.asarray(inputs["cond_w2"], f32)
    cb2 = np.asarray(inputs["cond_b2"], f32)
    blk_g = np.asarray(inputs["blk_ln_g"], f32)
    blk_b = np.asarray(inputs["blk_ln_b"], f32)
    blk_w1 = np.asarray(inputs["blk_w1"], f32)
    blk_w2 = np.asarray(inputs["blk_w2"], f32)
    blk_b1 = np.asarray(inputs["blk_b1"], f32)
    blk_b2 = np.asarray(inputs["blk_b2"], f32)

    # ---- weight-only folds ----
    q = (probe @ wq + bq) * RSQRT_DH
    U = np.zeros((D, H), f32)
    for h in range(H):
        U[:, h] = wk[:, h * DH:(h + 1) * DH] @ q[h * DH:(h + 1) * DH]
    U8 = (U * SU).astype(NP8)
    rin_cond = rin_w[0:TD]
    rin_pool = np.ascontiguousarray(rin_w[TD:TD + D])
    rin_na = rin_w[TD + D:]
    w2rin = w2 @ rin_pool
    cw2rin = cw2 @ rin_cond
    rb_fold = (rin_b + b2 @ rin_pool + cb2 @ rin_cond) / NC
    # LN affine folds: y_aff @ W = y_core @ (g*W) + b@W
    w1_aff = ln_g[:, None] * w1              # (D, 4D)
    b1_aff = b1 + ln_b @ w1                  # (4D,)
    bw1_aff = blk_g[:, :, None] * blk_w1     # (3, HID, 4HID)
    bb1_aff = blk_b1 + np.einsum('ih,ihf->if', blk_b, blk_w1)

    def ptile(m, c_per_g):
        K, N = m.shape
        G = K // (P * c_per_g)
        r = np.ascontiguousarray(
            m.reshape(G, c_per_g, P, N).transpose(0, 2, 1, 3))
        return r if G > 1 else r[0]

    shared = {
        "rp16": np.ascontiguousarray(
            rin_pool.reshape(DC, P, HID).transpose(1, 0, 2)).astype(f16),
        "four_w2": np.concatenate(
            [np.asarray(inputs["four_w"], f32).reshape(TD // 2, 1)] * 2),
        "phase2": np.concatenate(
            [np.full((TD // 2, 1), np.pi / 2, f32),
             np.zeros((TD // 2, 1), f32)]),
        "timeT": np.ascontiguousarray(np.asarray(inputs["time"], f32).T),
        "naT": np.ascontiguousarray(
            np.asarray(inputs["noisy_actions"], f32).T).astype(f16),
        "cw1": np.asarray(inputs["cond_w1"], f32).astype(f16),
        "cb1c": np.asarray(inputs["cond_b1"], f32).reshape(-1, 1),
        "cw2rin8": (cw2rin / NC).astype(f16),
        "rna8": (rin_na / NC).astype(f16),
        "rb8": rb_fold.astype(f16).reshape(1, HID),
        "bw1a": np.ascontiguousarray(
            bw1_aff.reshape(NBLK * HC, P, 4 * HID)[0:3].transpose(1, 0, 2)
        ).astype(f16),
        "bw1b": np.ascontiguousarray(
            bw1_aff.reshape(NBLK * HC, P, 4 * HID)[3:6].transpose(1, 0, 2)
        ).astype(f16),
        "bw2a": np.ascontiguousarray(
            blk_w2.reshape(NBLK * 8, P, HID)[0:12].transpose(1, 0, 2)
        ).astype(f16),
        "bw2b": np.ascontiguousarray(
            blk_w2.reshape(NBLK * 8, P, HID)[12:24].transpose(1, 0, 2)
        ).astype(f16),
        "ow": np.ascontiguousarray(
            np.asarray(inputs["out_w"], f32).reshape(HC, P, AD)
            .transpose(1, 0, 2)).astype(f16),
        "out_bc": np.asarray(inputs["out_b"], f32).reshape(1, AD),
        "U8r": np.ascontiguousarray(U8.reshape(DC, P, H).transpose(1, 0, 2)),
    }

    in_maps = []
    for i in range(NC):
        hb_ = slice(i * DH, (i + 1) * DH)
        fb = slice(i * F1S, (i + 1) * F1S)
        m = dict(shared)
        m["llm16"] = llm_full[i].astype(f16)
        m["llmT8"] = np.ascontiguousarray(llm_full[i].T).astype(NP8)
        m["wv16"] = ptile(np.ascontiguousarray(wv[:, hb_]), 4).astype(f16)
        m["wo16"] = np.ascontiguousarray(wo[hb_, :]).astype(f16)
        m["w116"] = ptile(np.ascontiguousarray(w1_aff[:, fb]), 2).astype(f16)
        m["w2rin16"] = ptile(np.ascontiguousarray(w2rin[fb]), FC).astype(f16)
        brep = np.zeros((1, BREP_N), f16)
        brep[0, BO_O:BO_O + D] = (bo / NC).astype(f16)
        brep[0, B1_O:B1_O + F1S] = b1_aff[fb].astype(f16)
        brep[0, BV_O:BV_O + DH] = bv[hb_].astype(f16)
        brep[0, BB1_O:BB1_O + NBLK * 4 * HID] = bb1_aff.reshape(-1).astype(f16)
        brep[0, BB2_O:BB2_O + NBLK * HID] = blk_b2.reshape(-1).astype(f16)
        m["brep"] = brep
        in_maps.append(m)
    return in_maps


def kernel(**inputs):
    nc = _get_nc()
    in_maps = _prep_in_maps(inputs)
    r = run_bass_kernel_spmd(nc, in_maps, core_ids=list(range(NC)))
    return np.ascontiguousarray(r.results[0]["res"]).astype(np.float32)


def run_traced(**inputs):
    nc = _get_nc()
    in_maps = _prep_in_maps(inputs)
    r = run_bass_kernel_spmd(nc, in_maps, core_ids=list(range(NC)), trace=True)
    return np.ascontiguousarray(r.results[0]["res"]).astype(np.float32), r
